# revision 16
# baseline (speedup 1.0000x reference)
"""Fused pre-LN + QKV + attention + post-LN + residual kernel for TRN2.

Problem (nn_Attention_86517821210894):
    B=2, N=4096, C=512, H=8, D=64
    xn  = LN(x) ; qkv = xn @ w_qkv + b ; per-(b,h) softmax attention
    val = LN(attn_out) ; out = xn + val

Sharding (8 cores, zero collectives):
    core c -> batch b = c // 4, query-row block r = c % 4 (1024 rows).
    Each core receives x[b] ROTATED so its query block is rows 0:1024
    (softmax and the value sum are permutation-invariant over keys),
    builds K/V for all 4096 keys, and produces out[b, r*1024:(r+1)*1024].

Design notes (v2, restructured from the 428us baseline using its trace):
  - ScalarE exp stream is the hard floor: 171 flushes x ~1.53us = 262us.
    Everything else is scheduled to keep that stream dense.
  - Score matmuls for the two heads of a pair are emitted ADJACENTLY so
    the PE runs them concurrently in the 64-row tiles (measured: a pair
    costs 512 cycles total, same as one solo MM).  The exp of a full
    PSUM group is emitted lazily - just before the next group alloc -
    so it never splits a pair.
  - w_qkv is DMA'd in 5 column-block pieces (K-pair0 and Q-pair0 first)
    so the first kT matmul no longer waits 14us for the full weight load.
  - v/kT/qT production is spread: kT(p+1) chunks ride the free pav PSUM
    banks during the first flushes of each stream (before AV claims
    them, av_start_flush=7/8); v rides the prefix ps3 ring just-in-time
    for the trailing AV.
  - Stream order interleaves qb: (p0,q0=prefix),(p0,q1),(p1,q0),... so
    each pair's kT production is split across the two preceding streams.
  - Tail: phase-5 post-LN work is split across DVE and GpSimd (Pool) and
    pipelined per 128-row tile to shrink the after-last-exp tail.
"""

import sys

sys.path.insert(0, "/opt/trn_rl_repo")

import numpy as np

B, N, C, H = 2, 4096, 512, 8
D = C // H
QR = N // 4  # query rows per core
EPS = 1e-5
SCALE = float(D) ** -0.5

_CACHE = {}


def _build(flags):
    (use_g_pre, use_beta_pre, use_g_post, use_beta_post, use_b_q, use_b_v) = flags

    import concourse.bacc as bacc
    import concourse.bass as bass
    import concourse.tile as tile
    from concourse import mybir
    from concourse.masks import make_identity

    f32 = mybir.dt.float32
    bf16 = mybir.dt.bfloat16
    AF = mybir.ActivationFunctionType
    ALU = mybir.AluOpType

    nc = bacc.Bacc(
        "TRN2", target_bir_lowering=False, debug=False, enable_asserts=False
    )

    xb = nc.dram_tensor("xb", [N, C], f32, kind="ExternalInput").ap()
    wk0_d = nc.dram_tensor("wk0", [128, C // 128, 128], bf16, kind="ExternalInput").ap()
    wq0_d = nc.dram_tensor("wq0", [128, C // 128, 128], bf16, kind="ExternalInput").ap()
    wv_d = nc.dram_tensor("wv", [128, C // 128, C], bf16, kind="ExternalInput").ap()
    wk1_d = nc.dram_tensor("wk1", [128, C // 128, 384], bf16, kind="ExternalInput").ap()
    wq1_d = nc.dram_tensor("wq1", [128, C // 128, 384], bf16, kind="ExternalInput").ap()
    bqkv = nc.dram_tensor("b_qkv", [3 * C], f32, kind="ExternalInput").ap()
    g_pre = nc.dram_tensor("g_pre", [C], f32, kind="ExternalInput").ap()
    beta_pre = nc.dram_tensor("beta_pre", [C], f32, kind="ExternalInput").ap()
    g_post = nc.dram_tensor("g_post", [C], f32, kind="ExternalInput").ap()
    beta_post = nc.dram_tensor("beta_post", [C], f32, kind="ExternalInput").ap()
    out = nc.dram_tensor("out", [QR, C], f32, kind="ExternalOutput").ap()

    NT = N // 128  # 32 row tiles of x[b]
    QT = QR // 128  # 8 row tiles of the query block
    CCH = C // 128  # 4 contraction chunks
    KC = N // 128  # 32 key chunks
    NPAIR = H // 2
    NQB = QR // 512  # 2 query blocks of 512

    def bcast(vec_ap, p):
        return bass.AP(
            tensor=vec_ap.tensor, offset=vec_ap.offset, ap=[[0, p], *vec_ap.ap]
        )

    with tile.TileContext(nc) as tc:
        with (
            tc.tile_pool(name="consts", bufs=1) as consts,
            tc.tile_pool(name="ln_in", bufs=6) as ln_in,
            tc.tile_pool(name="stats", bufs=8) as stats,
            tc.tile_pool(name="xnrow", bufs=1) as xnrow_pool,
            tc.tile_pool(name="xnT", bufs=1) as xnT_pool,
            tc.tile_pool(name="vsb", bufs=1) as v_pool,
            tc.tile_pool(name="kT", bufs=1) as kT_pool,
            tc.tile_pool(name="qT", bufs=1) as qT_pool,
            tc.tile_pool(name="expT", bufs=7) as expT_pool,
            tc.tile_pool(name="valT", bufs=2) as valT_pool,
            tc.tile_pool(name="valasm", bufs=1) as val_pool,
            tc.tile_pool(name="outp", bufs=4) as out_pool,
            tc.tile_pool(name="ps3", bufs=2, space="PSUM") as ps3,
            tc.tile_pool(name="pav", bufs=2, space="PSUM") as psum_av,
        ):
            # ---- warmup burst: bring the PE HAM to K=8/8 immediately ----
            dummy = consts.tile([128, 512], bf16)
            nc.vector.memset(dummy, 0.0)
            pw = ps3.tile([128, 3, 512], f32, tag="ps3")
            for _ in range(20):
                nc.tensor.matmul(pw[:, 0, :], dummy[:, 0:128], dummy)
            del pw

            # ---- constants ----
            ident = consts.tile([128, 128], f32)
            make_identity(nc, ident)
            ident_bf = consts.tile([128, 128], bf16)
            make_identity(nc, ident_bf)
            eps_t = consts.tile([128, 1], f32)
            nc.vector.memset(eps_t, EPS)
            seed_b = consts.tile([128, 1], f32)
            nc.vector.memset(seed_b, 0.5 * 0.6931471805599453 * 127.0)

            # ---- weights: host-prearranged blocks, contiguous DMAs ----
            w_k0 = consts.tile([128, CCH, 128], bf16)
            nc.sync.dma_start(out=w_k0, in_=wk0_d)
            w_q0 = consts.tile([128, CCH, 128], bf16)
            nc.sync.dma_start(out=w_q0, in_=wq0_d)
            w_v = consts.tile([128, CCH, C], bf16)
            w_k1 = consts.tile([128, CCH, 384], bf16)
            w_q1 = consts.tile([128, CCH, 384], bf16)

            g_pre_t = beta_pre_t = g_post_t = beta_post_t = None
            if use_g_pre:
                g_pre_t = consts.tile([128, C], f32)
                nc.sync.dma_start(out=g_pre_t, in_=bcast(g_pre, 128))
            if use_beta_pre:
                beta_pre_t = consts.tile([128, C], f32)
                nc.sync.dma_start(out=beta_pre_t, in_=bcast(beta_pre, 128))
            bq_t = None
            if use_b_q:
                bq_t = consts.tile([128, CCH, 1], f32)
                nc.sync.dma_start(
                    out=bq_t, in_=bqkv[0:C].rearrange("(cc p) -> p cc 1", p=128)
                )
            g_post_t = beta_post_t = bv_t = None

            def wk(pair, cc):
                if pair == 0:
                    return w_k0[:, cc, :]
                return w_k1[:, cc, (pair - 1) * 128 : pair * 128]

            def wq(pair, cc):
                if pair == 0:
                    return w_q0[:, cc, :]
                return w_q1[:, cc, (pair - 1) * 128 : pair * 128]

            # ---- persistent tensors ----
            xn_rows = xnrow_pool.tile([128, QT, C], f32)
            xnT = xnT_pool.tile([128, CCH, N], bf16)
            v_sb = v_pool.tile([128, KC, H, D + 1], bf16)
            val_asm = val_pool.tile([128, QT, H, D + 1], f32)
            kTs = [
                kT_pool.tile([128, N], bf16, tag="kT", bufs=4, name=f"kT{i}")
                for i in range(4)
            ]
            qTs = [
                qT_pool.tile([128, QR], bf16, tag="qT", bufs=4, name=f"qT{i}")
                for i in range(4)
            ]

            nc.vector.memset(v_sb[:, :, :, D : D + 1], 1.0)

            def rsqrt_into(dst, a4, w, tag):
                """dst = 1/sqrt(a4), a4 > 0, [128, w] f32 (Exp-seed + Newton)."""
                ai = a4.bitcast(mybir.dt.int32)
                fi = stats.tile([128, w], f32, tag=tag + "_f")
                nc.vector.tensor_copy(out=fi, in_=ai)
                nc.scalar.activation(
                    out=dst,
                    in_=fi,
                    func=AF.Exp,
                    scale=-0.5 * 0.6931471805599453 / 8388608.0,
                    bias=seed_b,
                )
                for _ in range(1):
                    t = stats.tile([128, w], f32, tag=tag + "_t")
                    nc.vector.tensor_mul(out=t, in0=dst, in1=dst)
                    nc.vector.tensor_mul(out=t, in0=t, in1=a4)
                    nc.vector.tensor_scalar(
                        out=t,
                        in0=t,
                        scalar1=-0.5,
                        scalar2=1.5,
                        op0=ALU.mult,
                        op1=ALU.add,
                    )
                    nc.vector.tensor_mul(out=dst, in0=dst, in1=t)

            def transpose_into(src, col0):
                ps = ps3.tile([128, 3, 1024], bf16, tag="ps3")
                pview = ps[:, 0, 0:512].rearrange("p (c n) -> p c n", n=128)
                for cc in range(CCH):
                    nc.tensor.transpose(
                        pview[:, cc, :], src[:, cc * 128 : (cc + 1) * 128], ident_bf
                    )
                nc.vector.tensor_copy(
                    out=xnT[:, :, col0 : col0 + 128], in_=pview
                )

            def produce_v(kc, eng=None):
                pv = ps3.tile([128, 3, 512], f32, tag="ps3")
                for cc in range(CCH):
                    nc.tensor.matmul(
                        pv[:, 0, :],
                        xnT[:, cc, kc * 128 : (kc + 1) * 128],
                        w_v[:, cc, :],
                        start=(cc == 0),
                        stop=(cc == CCH - 1),
                    )
                src = pv[:, 0, :].rearrange("p (h d) -> p h d", d=D)
                dst = v_sb[:, kc, :, 0:D]
                if use_b_v:
                    nc.vector.tensor_add(
                        out=dst, in0=src, in1=bv_t.rearrange("p (h d) -> p h d", d=D)
                    )
                elif eng is None:
                    nc.scalar.copy(out=dst, in_=src)
                else:
                    eng.tensor_copy(out=dst, in_=src)

            def produce_kT(pair, rc, on_pav=False, evac=None):
                kT = kTs[pair]
                if on_pav:
                    pk = psum_av.tile([128, 512], f32, tag="pav", name="pkv")
                else:
                    pk3 = ps3.tile([128, 3, 512], f32, tag="ps3", name="pk3")
                    pk = pk3[:, 0, :]
                for cc in range(CCH):
                    nc.tensor.matmul(
                        pk,
                        wk(pair, cc),
                        xnT[:, cc, rc * 512 : (rc + 1) * 512],
                        start=(cc == 0),
                        stop=(cc == CCH - 1),
                    )
                if evac is None:
                    nc.vector.tensor_copy(
                        out=kT[:, rc * 512 : (rc + 1) * 512], in_=pk
                    )
                else:
                    evac.copy(out=kT[:, rc * 512 : (rc + 1) * 512], in_=pk)

            def produce_qT(pair, rc, on_pav=False):
                qT = qTs[pair]
                if on_pav:
                    pq = psum_av.tile([128, 512], f32, tag="pav", name="pqv")
                else:
                    pq3 = ps3.tile([128, 3, 512], f32, tag="ps3", name="pq3")
                    pq = pq3[:, 0, :]
                for cc in range(CCH):
                    nc.tensor.matmul(
                        pq,
                        wq(pair, cc),
                        xnT[:, cc, rc * 512 : (rc + 1) * 512],
                        start=(cc == 0),
                        stop=(cc == CCH - 1),
                    )
                if use_b_q:
                    nc.vector.tensor_scalar_add(
                        out=qT[:, rc * 512 : (rc + 1) * 512],
                        in0=pq,
                        scalar1=bq_t[:, pair, :],
                    )
                else:
                    nc.vector.tensor_copy(
                        out=qT[:, rc * 512 : (rc + 1) * 512], in_=pq
                    )

            # ---- attention stream machinery ----
            class AttState:
                def __init__(self):
                    self.group = None  # (tile, pos, pending)
                    self.full = None
                    self.exp_of = {}
                    self.pavs = None
                    self.av_next = 0
                    self.fl = 0

            def do_flush(st):
                tile_, pos, pending = st.full
                ex = expT_pool.tile([128, 3, 512], bf16, tag="expT")
                nc.scalar.activation(
                    out=ex[:, 0:pos, :],
                    in_=tile_[:, 0:pos, :],
                    func=AF.Exp,
                    scale=SCALE,
                )
                for key, p in pending:
                    st.exp_of[key] = (ex, p)
                st.full = None
                st.fl += 1

            def emit_slice(st, pair, qb, kc, h_idx):
                if st.group is None:
                    if st.full is not None:
                        do_flush(st)
                    st.group = [
                        ps3.tile([128, 3, 512], f32, tag="ps3", name="grp"),
                        0,
                        [],
                    ]
                g = st.group
                base = h_idx * 64
                kT, qT = kTs[pair], qTs[pair]
                nc.tensor.matmul(
                    g[0][:, g[1], :],
                    kT[base : base + 64, kc * 128 : (kc + 1) * 128],
                    qT[base : base + 64, qb * 512 : (qb + 1) * 512],
                )
                g[2].append(((kc, h_idx), g[1]))
                g[1] += 1
                if g[1] == 3:
                    st.full = (g[0], 3, g[2])
                    st.group = None

            def av_drain(st, pair, cap=None, upto=None):
                if st.pavs is None:
                    pav_lo = psum_av.tile([128, 512], f32, tag="pav")
                    pav_hi = psum_av.tile([128, 512], f32, tag="pav")
                    st.pavs = (pav_lo, pav_hi)
                n = 0
                while (
                    st.av_next < KC
                    and (st.av_next, 0) in st.exp_of
                    and (st.av_next, 1) in st.exp_of
                ):
                    if upto is not None and st.av_next >= upto:
                        break
                    if cap is not None and n >= cap:
                        break
                    kc = st.av_next
                    for h_idx in range(2):
                        ex, p = st.exp_of.pop((kc, h_idx))
                        nc.tensor.matmul(
                            st.pavs[h_idx][0 : D + 1, :],
                            v_sb[:, kc, 2 * pair + h_idx, :],
                            ex[:, p, :],
                            start=(kc == 0),
                            stop=(kc == KC - 1),
                        )
                    st.av_next += 1
                    n += 1

            def run_stream(
                st, pair, qb, kcs, extras=None, av_start=0, av_cap=None,
                av_upto=None, flush_end=False,
            ):
                extras = dict(extras or {})
                for kc in kcs:
                    fl0 = st.fl
                    emit_slice(st, pair, qb, kc, 0)
                    emit_slice(st, pair, qb, kc, 1)
                    if st.full is not None:
                        do_flush(st)
                    if st.fl > fl0:
                        if st.fl in extras:
                            extras.pop(st.fl)()
                        if st.fl >= av_start:
                            cap = (
                                av_cap[st.fl % 2]
                                if isinstance(av_cap, tuple)
                                else av_cap
                            )
                            av_drain(st, pair, cap=cap, upto=av_upto)
                for k in sorted(extras):
                    extras.pop(k)()
                if flush_end and st.group is not None:
                    # never leave a partial group holding a ps3 ring slot
                    # across other ps3 users (prefix transposes/kT/v)
                    st.full = (st.group[0], st.group[1], st.group[2])
                    st.group = None
                    do_flush(st)
                    if st.fl >= av_start:
                        cap = (
                            av_cap[st.fl % 2] if isinstance(av_cap, tuple) else av_cap
                        )
                        av_drain(st, pair, cap=cap, upto=av_upto)

            def close_stream(st, pair):
                if st.group is not None:
                    st.full = (st.group[0], st.group[1], st.group[2])
                    st.group = None
                if st.full is not None:
                    do_flush(st)
                av_drain(st, pair)
                assert st.av_next == KC, st.av_next
                vts = []
                for h_idx in range(2):
                    vt = valT_pool.tile([D + 1, 512], f32, tag="valT", bufs=4)
                    nc.vector.tensor_copy(out=vt, in_=st.pavs[h_idx][0 : D + 1, :])
                    vts.append(vt)
                st.pavs = None
                return vts

            def transpose_half(vts, qb, pair, half, on_pav=False):
                h = 2 * pair + half
                if on_pav:
                    bank = psum_av.tile([128, 512], f32, tag="pav", name="tsc")
                else:
                    pt = ps3.tile([128, 3, 512], f32, tag="ps3")
                    bank = pt[:, 0, :]
                pv = bank.rearrange("p (j d) -> p j d", d=128)
                for j in range(4):
                    nc.tensor.transpose(
                        pv[:, j, 0 : D + 1],
                        vts[half][:, j * 128 : (j + 1) * 128],
                        ident[0 : D + 1, 0 : D + 1],
                    )
                nc.vector.tensor_copy(
                    out=val_asm[:, qb * 4 : qb * 4 + 4, h, :],
                    in_=pv[:, :, 0 : D + 1],
                )

            # ---- phase 5 per qb half ----
            class Ph5:
                def __init__(self, qtiles):
                    self.qtiles = qtiles
                    self.ots = []
                    self.m4 = stats.tile([128, 4], f32, tag="m4b", name="m4")
                    self.a4b = stats.tile([128, 4], f32, tag="a4b", name="a4b")
                    self.r4 = stats.tile([128, 4], f32, tag="r4b", name="r4")

                def stats_one(self, jj, eng=None, mul_act=False):
                    eng = eng or nc.vector
                    qtile = self.qtiles[jj]
                    va = val_asm[:, qtile]
                    ot = out_pool.tile([128, C], f32, tag="ot", name="ot")
                    rs8 = stats.tile([128, 8], f32, tag="rs8", name="rs8")
                    nc.vector.reciprocal(
                        out=rs8,
                        in_=va[:, :, D : D + 1].rearrange("p h one -> p (h one)"),
                    )
                    for h in range(H):
                        if mul_act:
                            nc.scalar.activation(
                                out=ot[:, h * D : (h + 1) * D],
                                in_=va[:, h, 0:D],
                                func=AF.Identity,
                                scale=rs8[:, h : h + 1],
                            )
                        else:
                            eng.tensor_scalar_mul(
                                out=ot[:, h * D : (h + 1) * D],
                                in0=va[:, h, 0:D],
                                scalar1=rs8[:, h : h + 1],
                            )
                    if use_b_v:
                        eng.tensor_add(out=ot, in0=ot, in1=bv_t)
                    st6 = stats.tile([128, 6], f32, tag="bn6", name="st6")
                    nc.vector.bn_stats(out=st6, in_=ot)
                    mv = stats.tile([128, 2], f32, tag="mv", name="mv")
                    nc.vector.bn_aggr(out=mv, in_=st6)
                    nc.vector.tensor_copy(out=self.m4[:, jj : jj + 1], in_=mv[:, 0:1])
                    nc.vector.tensor_copy(out=self.a4b[:, jj : jj + 1], in_=mv[:, 1:2])
                    self.ots.append(ot)

                def rsqrt(self):
                    nc.vector.tensor_scalar_add(out=self.a4b, in0=self.a4b, scalar1=EPS)
                    rsqrt_into(self.r4, self.a4b, 4, "p5")
                    self.nmr = stats.tile([128, 4], f32, tag="nmr", name="nmr")
                    nc.vector.tensor_mul(out=self.nmr, in0=self.m4, in1=self.r4)
                    nc.vector.tensor_scalar_mul(out=self.nmr, in0=self.nmr, scalar1=-1.0)

                def fin_one(self, jj, eng=None, use_act=False):
                    eng = eng or nc.vector
                    qtile = self.qtiles[jj]
                    ot = self.ots[jj]
                    if use_act:
                        # (x - m)*r on ScalarE (idle at the tail): x*r - m*r
                        nc.scalar.activation(
                            out=ot,
                            in_=ot,
                            func=AF.Identity,
                            scale=self.r4[:, jj : jj + 1],
                            bias=self.nmr[:, jj : jj + 1],
                        )
                    else:
                        eng.tensor_scalar(
                            out=ot,
                            in0=ot,
                            scalar1=self.m4[:, jj : jj + 1],
                            scalar2=self.r4[:, jj : jj + 1],
                            op0=ALU.subtract,
                            op1=ALU.mult,
                        )
                    if use_g_post:
                        eng.tensor_mul(out=ot, in0=ot, in1=g_post_t)
                    if use_beta_post:
                        eng.tensor_add(out=ot, in0=ot, in1=beta_post_t)
                    eng.tensor_add(out=ot, in0=ot, in1=xn_rows[:, qtile, :])
                    nc.sync.dma_start(
                        out=out[qtile * 128 : (qtile + 1) * 128, :], in_=ot
                    )

            # ================= emission =================
            states = [[AttState() for _ in range(NQB)] for _ in range(NPAIR)]
            st00 = states[0][0]

            # phase 1+2: LN -> xnT transposes -> kT0/qT0 -> v (lagged),
            # with the pair0-qb0 score/exp/AV stream trickling one rc behind
            for rc in range(NT // 4):
                if rc >= 1:
                    run_stream(
                        st00, 0, 0, range(4 * (rc - 1), 4 * rc),
                        av_upto=4 * (rc - 1), flush_end=True,
                    )
                xts, mvs = [], []
                for j in range(4):
                    i = 4 * rc + j
                    xt = ln_in.tile([128, C], f32, tag="xt")
                    nc.sync.dma_start(out=xt, in_=xb[i * 128 : (i + 1) * 128, :])
                    st6 = stats.tile([128, 6], f32, tag="bn6")
                    nc.vector.bn_stats(out=st6, in_=xt)
                    mv = stats.tile([128, 2], f32, tag="mv")
                    nc.vector.bn_aggr(out=mv, in_=st6)
                    xts.append(xt)
                    mvs.append(mv)
                a4 = stats.tile([128, 4], f32, tag="a4")
                for j in range(4):
                    nc.vector.tensor_copy(out=a4[:, j : j + 1], in_=mvs[j][:, 1:2])
                nc.vector.tensor_scalar_add(out=a4, in0=a4, scalar1=EPS)
                r4 = stats.tile([128, 4], f32, tag="r4")
                rsqrt_into(r4, a4, 4, "p1")
                for j in range(4):
                    i = 4 * rc + j
                    eng = nc.vector
                    xbf = ln_in.tile([128, C], bf16, tag="xbf", bufs=4)
                    if i < QT or use_g_pre or use_beta_pre:
                        dst = xn_rows[:, i, :] if i < QT else xts[j]
                        eng.tensor_scalar(
                            out=dst,
                            in0=xts[j],
                            scalar1=mvs[j][:, 0:1],
                            scalar2=r4[:, j : j + 1],
                            op0=ALU.subtract,
                            op1=ALU.mult,
                        )
                        if use_g_pre:
                            eng.tensor_mul(out=dst, in0=dst, in1=g_pre_t)
                        if use_beta_pre:
                            eng.tensor_add(out=dst, in0=dst, in1=beta_pre_t)
                        eng.tensor_copy(out=xbf, in_=dst)
                    else:
                        eng.tensor_scalar(
                            out=xbf,
                            in0=xts[j],
                            scalar1=mvs[j][:, 0:1],
                            scalar2=r4[:, j : j + 1],
                            op0=ALU.subtract,
                            op1=ALU.mult,
                        )
                    transpose_into(xbf, i * 128)
                produce_kT(0, rc)
                if rc < NQB:
                    produce_qT(0, rc)
                # late prefix: next pair's kT ch0-3 on the ring
                if rc >= 6:
                    for r2 in range(2 * (rc - 6), 2 * (rc - 6) + 2):
                        produce_kT(1, r2)
                # v lags one rc: the trickle AV (bounded by av_upto) only
                # touches kc < 4*(rc-1), produced in the previous iteration
                if rc >= 1:
                    for kc in range(4 * (rc - 1), 4 * rc):
                        produce_v(kc)
                if rc == 0:
                    # bulk weight DMAs, after the first x tiles are queued
                    nc.sync.dma_start(out=w_v, in_=wv_d)
                    nc.sync.dma_start(out=w_k1, in_=wk1_d)
                    nc.sync.dma_start(out=w_q1, in_=wq1_d)
                    if use_b_v:
                        bv_t = consts.tile([128, C], f32)
                        nc.sync.dma_start(
                            out=bv_t, in_=bcast(bqkv[2 * C : 3 * C], 128)
                        )
                    if use_g_post:
                        g_post_t = consts.tile([128, C], f32)
                        nc.sync.dma_start(out=g_post_t, in_=bcast(g_post, 128))
                    if use_beta_post:
                        beta_post_t = consts.tile([128, C], f32)
                        nc.sync.dma_start(
                            out=beta_post_t, in_=bcast(beta_post, 128)
                        )
                # late prefix: next pair's kT ch0-3 on the ring
                if rc >= 6:
                    for r2 in range(2 * (rc - 6), 2 * (rc - 6) + 2):
                        produce_kT(1, r2)
                # v lags one rc: the trickle AV (bounded by av_upto) only
                # touches kc < 4*(rc-1), produced in the previous iteration
                if rc >= 1:
                    for kc in range(4 * (rc - 1), 4 * rc):
                        produce_v(kc)
                if rc == 0:
                    # bulk weight DMAs, after the first x tiles are queued
                    nc.sync.dma_start(out=w_v, in_=wv_d)
                    nc.sync.dma_start(out=w_k1, in_=wk1_d)
                    nc.sync.dma_start(out=w_q1, in_=wq1_d)
                    if use_b_v:
                        bv_t = consts.tile([128, C], f32)
                        nc.sync.dma_start(
                            out=bv_t, in_=bcast(bqkv[2 * C : 3 * C], 128)
                        )
                    if use_g_post:
                        g_post_t = consts.tile([128, C], f32)
                        nc.sync.dma_start(out=g_post_t, in_=bcast(g_post, 128))
                    if use_beta_post:
                        beta_post_t = consts.tile([128, C], f32)
                        nc.sync.dma_start(
                            out=beta_post_t, in_=bcast(beta_post, 128)
                        )
                if 2 <= rc <= 4:
                    # qT pair rc-1, both chunks in ONE ring slot
                    pq2 = ps3.tile([128, 3, 512], f32, tag="ps3", name="pq2")
                    for i in range(2):
                        for cc in range(CCH):
                            nc.tensor.matmul(
                                pq2[:, i, :],
                                wq(rc - 1, cc),
                                xnT[:, cc, i * 512 : (i + 1) * 512],
                                start=(cc == 0),
                                stop=(cc == CCH - 1),
                            )
                    for i in range(2):
                        if use_b_q:
                            nc.vector.tensor_scalar_add(
                                out=qTs[rc - 1][:, i * 512 : (i + 1) * 512],
                                in0=pq2[:, i, :],
                                scalar1=bq_t[:, rc - 1, :],
                            )
                        else:
                            nc.vector.tensor_copy(
                                out=qTs[rc - 1][:, i * 512 : (i + 1) * 512],
                                in_=pq2[:, i, :],
                            )

            # stream0 tail: last v chunks, remaining scores, full AV drain
            for kc in range(28, 32):
                produce_v(kc, eng=nc.vector)
            run_stream(st00, 0, 0, range(28, 32))
            vts_prev = close_stream(st00, 0)
            tp_prev = (0, 0)

            # streams 1..7, qb-interleaved: (p0,q1),(p1,q0),(p1,q1),...
            order = [(0, 1), (1, 0), (1, 1), (2, 0), (2, 1), (3, 0), (3, 1)]
            ph5a = Ph5([0, 1, 2, 3])
            for pair, qb in order:
                st = states[pair][qb]
                pp, pqb = tp_prev
                vp = vts_prev
                ex = {
                    1: (lambda vp=vp, pqb=pqb, pp=pp: transpose_half(vp, pqb, pp, 0, on_pav=True)),
                    2: (lambda vp=vp, pqb=pqb, pp=pp: transpose_half(vp, pqb, pp, 1, on_pav=True)),
                }
                av_start = 7
                if qb == 0 and pair + 1 < NPAIR:
                    for t in range(4):
                        ex[3 + t] = (
                            lambda pair=pair, t=t: produce_kT(pair + 1, t, on_pav=True)
                        )
                elif qb == 1 and pair + 1 < NPAIR:
                    for t in range(4):
                        ex[3 + t] = (
                            lambda pair=pair, t=t: produce_kT(
                                pair + 1, 4 + t, on_pav=True
                            )
                        )
                    ex[7] = lambda pair=pair: produce_qT(pair + 1, 0, on_pav=True)
                    ex[8] = lambda pair=pair: produce_qT(pair + 1, 1, on_pav=True)
                    av_start = 9
                if (pair, qb) == (3, 1):
                    # phase5 for the qb0 rows rides the last stream
                    ex.update(
                        {
                            4: lambda: ph5a.stats_one(0),
                            6: lambda: ph5a.stats_one(1),
                            8: lambda: ph5a.stats_one(2),
                            10: lambda: ph5a.stats_one(3),
                            12: lambda: ph5a.rsqrt(),
                            14: lambda: ph5a.fin_one(0),
                            16: lambda: ph5a.fin_one(1),
                            18: lambda: ph5a.fin_one(2),
                            20: lambda: ph5a.fin_one(3),
                        }
                    )
                run_stream(
                    st, pair, qb, range(KC),
                    extras=ex, av_start=av_start, av_cap=5,
                )
                vts_prev = close_stream(st, pair)
                tp_prev = (pair, qb)

            # tail: last vt transposes + phase5 for qb1, DVE/Pool pipelined
            transpose_half(vts_prev, tp_prev[1], tp_prev[0], 0, on_pav=True)
            transpose_half(vts_prev, tp_prev[1], tp_prev[0], 1, on_pav=True)
            ph5b = Ph5([4, 5, 6, 7])
            ph5b.stats_one(0, mul_act=True)
            ph5b.stats_one(1)
            ph5b.stats_one(2, mul_act=True)
            ph5b.stats_one(3)
            ph5b.rsqrt()
            ph5b.fin_one(0, use_act=True)
            ph5b.fin_one(1)
            ph5b.fin_one(2, use_act=True)
            ph5b.fin_one(3)

    nc.compile()
    return nc


def kernel(x, w_qkv, b_qkv, g_pre, beta_pre, g_post, beta_post):
    import ml_dtypes
    from concourse.bass_utils import run_bass_kernel_spmd

    x = np.asarray(x, dtype=np.float32)
    w_qkv = np.asarray(w_qkv, dtype=np.float32)
    b_qkv = np.asarray(b_qkv, dtype=np.float32)
    g_pre = np.asarray(g_pre, dtype=np.float32)
    beta_pre = np.asarray(beta_pre, dtype=np.float32)
    g_post = np.asarray(g_post, dtype=np.float32)
    beta_post = np.asarray(beta_post, dtype=np.float32)

    flags = (
        not np.all(g_pre == 1.0),
        not np.all(beta_pre == 0.0),
        not np.all(g_post == 1.0),
        not np.all(beta_post == 0.0),
        not np.all(b_qkv[0:C] == 0.0),
        not np.all(b_qkv[2 * C : 3 * C] == 0.0),
    )
    # NOTE: b_qkv[C:2C] (the K bias) provably cancels in softmax and is
    # intentionally never applied.
    if flags not in _CACHE:
        _CACHE[flags] = _build(flags)
    nc = _CACHE[flags]

    w_bf = w_qkv.astype(ml_dtypes.bfloat16)
    wt = w_bf.reshape(4, 128, 3 * C)  # (cc, p, m)

    def blk(lo, hi):
        return np.ascontiguousarray(wt[:, :, lo:hi].transpose(1, 0, 2))

    wk0 = blk(C, C + 128)
    wq0 = blk(0, 128)
    wv = blk(2 * C, 3 * C)
    wk1 = blk(C + 128, 2 * C)
    wq1 = blk(128, C)
    in_maps = []
    for c in range(8):
        b = c // 4
        r = c % 4
        xrot = np.ascontiguousarray(
            np.concatenate([x[b, r * QR :], x[b, : r * QR]], axis=0)
        )
        in_maps.append(
            {
                "xb": xrot,
                "wk0": wk0,
                "wq0": wq0,
                "wv": wv,
                "wk1": wk1,
                "wq1": wq1,
                "b_qkv": b_qkv,
                "g_pre": g_pre,
                "beta_pre": beta_pre,
                "g_post": g_post,
                "beta_post": beta_post,
            }
        )

    global _last_in_maps
    _last_in_maps = in_maps
    res = run_bass_kernel_spmd(nc, in_maps, core_ids=list(range(8)))
    out = np.empty((B, N, C), dtype=np.float32)
    for c in range(8):
        b = c // 4
        r = c % 4
        out[b, r * QR : (r + 1) * QR] = res.results[c]["out"]
    return out


# revision 18
# speedup vs baseline: 1.0799x; 1.0799x over previous
"""Fused pre-LN + QKV + attention + post-LN + residual kernel for TRN2.

Problem (nn_Attention_86517821210894):
    B=2, N=4096, C=512, H=8, D=64
    xn  = LN(x) ; qkv = xn @ w_qkv + b ; per-(b,h) softmax attention
    val = LN(attn_out) ; out = xn + val

Sharding (8 cores, zero collectives):
    core c -> batch b = c // 4, query-row block r = c % 4 (1024 rows).
    Each core receives x[b] ROTATED so its query block is rows 0:1024
    (softmax and the value sum are permutation-invariant over keys),
    builds K/V for all 4096 keys, and produces out[b, r*1024:(r+1)*1024].

Design notes (v2, restructured from the 428us baseline using its trace):
  - ScalarE exp stream is the hard floor: 171 flushes x ~1.53us = 262us.
    Everything else is scheduled to keep that stream dense.
  - Score matmuls for the two heads of a pair are emitted ADJACENTLY so
    the PE runs them concurrently in the 64-row tiles (measured: a pair
    costs 512 cycles total, same as one solo MM).  The exp of a full
    PSUM group is emitted lazily - just before the next group alloc -
    so it never splits a pair.
  - w_qkv is DMA'd in 5 column-block pieces (K-pair0 and Q-pair0 first)
    so the first kT matmul no longer waits 14us for the full weight load.
  - v/kT/qT production is spread: kT(p+1) chunks ride the free pav PSUM
    banks during the first flushes of each stream (before AV claims
    them, av_start_flush=7/8); v rides the prefix ps3 ring just-in-time
    for the trailing AV.
  - Stream order interleaves qb: (p0,q0=prefix),(p0,q1),(p1,q0),... so
    each pair's kT production is split across the two preceding streams.
  - Tail: phase-5 post-LN work is split across DVE and GpSimd (Pool) and
    pipelined per 128-row tile to shrink the after-last-exp tail.
"""

import sys

sys.path.insert(0, "/opt/trn_rl_repo")

import numpy as np

B, N, C, H = 2, 4096, 512, 8
D = C // H
QR = N // 4  # query rows per core
EPS = 1e-5
SCALE = float(D) ** -0.5

_CACHE = {}


def _build(flags):
    (use_g_pre, use_beta_pre, use_g_post, use_beta_post, use_b_q, use_b_v) = flags

    import concourse.bacc as bacc
    import concourse.bass as bass
    import concourse.tile as tile
    from concourse import mybir
    from concourse.masks import make_identity

    f32 = mybir.dt.float32
    bf16 = mybir.dt.bfloat16
    AF = mybir.ActivationFunctionType
    ALU = mybir.AluOpType

    nc = bacc.Bacc(
        "TRN2", target_bir_lowering=False, debug=False, enable_asserts=False
    )

    xb = nc.dram_tensor("xb", [N, C], f32, kind="ExternalInput").ap()
    wk0_d = nc.dram_tensor("wk0", [128, C // 128, 128], bf16, kind="ExternalInput").ap()
    wq0_d = nc.dram_tensor("wq0", [128, C // 128, 128], bf16, kind="ExternalInput").ap()
    wv_d = nc.dram_tensor("wv", [128, C // 128, C], bf16, kind="ExternalInput").ap()
    wk1_d = nc.dram_tensor("wk1", [128, C // 128, 384], bf16, kind="ExternalInput").ap()
    wq1_d = nc.dram_tensor("wq1", [128, C // 128, 384], bf16, kind="ExternalInput").ap()
    bqkv = nc.dram_tensor("b_qkv", [3 * C], f32, kind="ExternalInput").ap()
    g_pre = nc.dram_tensor("g_pre", [C], f32, kind="ExternalInput").ap()
    beta_pre = nc.dram_tensor("beta_pre", [C], f32, kind="ExternalInput").ap()
    g_post = nc.dram_tensor("g_post", [C], f32, kind="ExternalInput").ap()
    beta_post = nc.dram_tensor("beta_post", [C], f32, kind="ExternalInput").ap()
    out = nc.dram_tensor("out", [QR, C], f32, kind="ExternalOutput").ap()

    NT = N // 128  # 32 row tiles of x[b]
    QT = QR // 128  # 8 row tiles of the query block
    CCH = C // 128  # 4 contraction chunks
    KC = N // 128  # 32 key chunks
    NPAIR = H // 2
    NQB = QR // 512  # 2 query blocks of 512

    def bcast(vec_ap, p):
        return bass.AP(
            tensor=vec_ap.tensor, offset=vec_ap.offset, ap=[[0, p], *vec_ap.ap]
        )

    with tile.TileContext(nc) as tc:
        with (
            tc.tile_pool(name="consts", bufs=1) as consts,
            tc.tile_pool(name="ln_in", bufs=6) as ln_in,
            tc.tile_pool(name="stats", bufs=8) as stats,
            tc.tile_pool(name="xnrow", bufs=1) as xnrow_pool,
            tc.tile_pool(name="xnT", bufs=1) as xnT_pool,
            tc.tile_pool(name="vsb", bufs=1) as v_pool,
            tc.tile_pool(name="kT", bufs=1) as kT_pool,
            tc.tile_pool(name="qT", bufs=1) as qT_pool,
            tc.tile_pool(name="expT", bufs=7) as expT_pool,
            tc.tile_pool(name="valT", bufs=2) as valT_pool,
            tc.tile_pool(name="valasm", bufs=1) as val_pool,
            tc.tile_pool(name="outp", bufs=4) as out_pool,
            tc.tile_pool(name="ps3", bufs=2, space="PSUM") as ps3,
            tc.tile_pool(name="pav", bufs=2, space="PSUM") as psum_av,
        ):
            # ---- warmup burst: bring the PE HAM to K=8/8 immediately ----
            dummy = consts.tile([128, 512], bf16)
            nc.vector.memset(dummy, 0.0)
            pw = ps3.tile([128, 3, 512], f32, tag="ps3")
            for _ in range(20):
                nc.tensor.matmul(pw[:, 0, :], dummy[:, 0:128], dummy)
            del pw

            # ---- constants ----
            ident = consts.tile([128, 128], f32)
            make_identity(nc, ident)
            ident_bf = consts.tile([128, 128], bf16)
            make_identity(nc, ident_bf)
            eps_t = consts.tile([128, 1], f32)
            nc.vector.memset(eps_t, EPS)
            seed_b = consts.tile([128, 1], f32)
            nc.vector.memset(seed_b, 0.5 * 0.6931471805599453 * 127.0)

            # ---- weights: host-prearranged blocks, contiguous DMAs ----
            w_k0 = consts.tile([128, CCH, 128], bf16)
            nc.sync.dma_start(out=w_k0, in_=wk0_d)
            w_q0 = consts.tile([128, CCH, 128], bf16)
            nc.sync.dma_start(out=w_q0, in_=wq0_d)
            w_v = consts.tile([128, CCH, C], bf16)
            w_k1 = consts.tile([128, CCH, 384], bf16)
            w_q1 = consts.tile([128, CCH, 384], bf16)

            g_pre_t = beta_pre_t = g_post_t = beta_post_t = None
            if use_g_pre:
                g_pre_t = consts.tile([128, C], f32)
                nc.sync.dma_start(out=g_pre_t, in_=bcast(g_pre, 128))
            if use_beta_pre:
                beta_pre_t = consts.tile([128, C], f32)
                nc.sync.dma_start(out=beta_pre_t, in_=bcast(beta_pre, 128))
            bq_t = None
            if use_b_q:
                bq_t = consts.tile([128, CCH, 1], f32)
                nc.sync.dma_start(
                    out=bq_t, in_=bqkv[0:C].rearrange("(cc p) -> p cc 1", p=128)
                )
            g_post_t = beta_post_t = bv_t = None

            def wk(pair, cc):
                if pair == 0:
                    return w_k0[:, cc, :]
                return w_k1[:, cc, (pair - 1) * 128 : pair * 128]

            def wq(pair, cc):
                if pair == 0:
                    return w_q0[:, cc, :]
                return w_q1[:, cc, (pair - 1) * 128 : pair * 128]

            # ---- persistent tensors ----
            xn_rows = xnrow_pool.tile([128, QT, C], f32)
            xnT = xnT_pool.tile([128, CCH, N], bf16)
            v_sb = v_pool.tile([128, KC, H, D + 1], bf16)
            val_asm = val_pool.tile([128, QT, H, D + 1], f32)
            kTs = [
                kT_pool.tile([128, N], bf16, tag="kT", bufs=4, name=f"kT{i}")
                for i in range(4)
            ]
            qTs = [
                qT_pool.tile([128, QR], bf16, tag="qT", bufs=4, name=f"qT{i}")
                for i in range(4)
            ]

            nc.vector.memset(v_sb[:, :, :, D : D + 1], 1.0)

            def rsqrt_into(dst, a4, w, tag):
                """dst = 1/sqrt(a4), a4 > 0, [128, w] f32 (Exp-seed + Newton)."""
                ai = a4.bitcast(mybir.dt.int32)
                fi = stats.tile([128, w], f32, tag=tag + "_f")
                nc.vector.tensor_copy(out=fi, in_=ai)
                nc.scalar.activation(
                    out=dst,
                    in_=fi,
                    func=AF.Exp,
                    scale=-0.5 * 0.6931471805599453 / 8388608.0,
                    bias=seed_b,
                )
                for _ in range(1):
                    t = stats.tile([128, w], f32, tag=tag + "_t")
                    nc.vector.tensor_mul(out=t, in0=dst, in1=dst)
                    nc.vector.tensor_mul(out=t, in0=t, in1=a4)
                    nc.vector.tensor_scalar(
                        out=t,
                        in0=t,
                        scalar1=-0.5,
                        scalar2=1.5,
                        op0=ALU.mult,
                        op1=ALU.add,
                    )
                    nc.vector.tensor_mul(out=dst, in0=dst, in1=t)

            def transpose_into(src, col0):
                ps = ps3.tile([128, 3, 1024], bf16, tag="ps3")
                pview = ps[:, 0, 0:512].rearrange("p (c n) -> p c n", n=128)
                for cc in range(CCH):
                    nc.tensor.transpose(
                        pview[:, cc, :], src[:, cc * 128 : (cc + 1) * 128], ident_bf
                    )
                nc.vector.tensor_copy(
                    out=xnT[:, :, col0 : col0 + 128], in_=pview
                )

            def produce_v(kc, eng=None):
                pv = ps3.tile([128, 3, 512], f32, tag="ps3")
                for cc in range(CCH):
                    nc.tensor.matmul(
                        pv[:, 0, :],
                        xnT[:, cc, kc * 128 : (kc + 1) * 128],
                        w_v[:, cc, :],
                        start=(cc == 0),
                        stop=(cc == CCH - 1),
                    )
                src = pv[:, 0, :].rearrange("p (h d) -> p h d", d=D)
                dst = v_sb[:, kc, :, 0:D]
                if use_b_v:
                    nc.vector.tensor_add(
                        out=dst, in0=src, in1=bv_t.rearrange("p (h d) -> p h d", d=D)
                    )
                elif eng is None:
                    nc.scalar.copy(out=dst, in_=src)
                else:
                    eng.tensor_copy(out=dst, in_=src)

            def produce_kT(pair, rc, on_pav=False, evac=None):
                kT = kTs[pair]
                if on_pav:
                    pk = psum_av.tile([128, 512], f32, tag="pav", name="pkv")
                else:
                    pk3 = ps3.tile([128, 3, 512], f32, tag="ps3", name="pk3")
                    pk = pk3[:, 0, :]
                for cc in range(CCH):
                    nc.tensor.matmul(
                        pk,
                        wk(pair, cc),
                        xnT[:, cc, rc * 512 : (rc + 1) * 512],
                        start=(cc == 0),
                        stop=(cc == CCH - 1),
                    )
                if evac is None:
                    nc.vector.tensor_copy(
                        out=kT[:, rc * 512 : (rc + 1) * 512], in_=pk
                    )
                else:
                    evac.copy(out=kT[:, rc * 512 : (rc + 1) * 512], in_=pk)

            def produce_qT(pair, rc, on_pav=False):
                qT = qTs[pair]
                if on_pav:
                    pq = psum_av.tile([128, 512], f32, tag="pav", name="pqv")
                else:
                    pq3 = ps3.tile([128, 3, 512], f32, tag="ps3", name="pq3")
                    pq = pq3[:, 0, :]
                for cc in range(CCH):
                    nc.tensor.matmul(
                        pq,
                        wq(pair, cc),
                        xnT[:, cc, rc * 512 : (rc + 1) * 512],
                        start=(cc == 0),
                        stop=(cc == CCH - 1),
                    )
                if use_b_q:
                    nc.vector.tensor_scalar_add(
                        out=qT[:, rc * 512 : (rc + 1) * 512],
                        in0=pq,
                        scalar1=bq_t[:, pair, :],
                    )
                else:
                    nc.vector.tensor_copy(
                        out=qT[:, rc * 512 : (rc + 1) * 512], in_=pq
                    )

            # ---- attention stream machinery ----
            class AttState:
                def __init__(self):
                    self.group = None  # (tile, pos, pending)
                    self.full = None
                    self.exp_of = {}
                    self.pavs = None
                    self.av_next = 0
                    self.fl = 0

            def do_flush(st):
                tile_, pos, pending = st.full
                ex = expT_pool.tile([128, 3, 512], bf16, tag="expT")
                nc.scalar.activation(
                    out=ex[:, 0:pos, :],
                    in_=tile_[:, 0:pos, :],
                    func=AF.Exp,
                    scale=SCALE,
                )
                for key, p in pending:
                    st.exp_of[key] = (ex, p)
                st.full = None
                st.fl += 1

            def emit_slice(st, pair, qb, kc, h_idx):
                if st.group is None:
                    if st.full is not None:
                        do_flush(st)
                    st.group = [
                        ps3.tile([128, 3, 512], f32, tag="ps3", name="grp"),
                        0,
                        [],
                    ]
                g = st.group
                base = h_idx * 64
                kT, qT = kTs[pair], qTs[pair]
                nc.tensor.matmul(
                    g[0][:, g[1], :],
                    kT[base : base + 64, kc * 128 : (kc + 1) * 128],
                    qT[base : base + 64, qb * 512 : (qb + 1) * 512],
                )
                g[2].append(((kc, h_idx), g[1]))
                g[1] += 1
                if g[1] == 3:
                    st.full = (g[0], 3, g[2])
                    st.group = None

            def av_drain(st, pair, cap=None, upto=None):
                if st.pavs is None:
                    pav_lo = psum_av.tile([128, 512], f32, tag="pav")
                    pav_hi = psum_av.tile([128, 512], f32, tag="pav")
                    st.pavs = (pav_lo, pav_hi)
                n = 0
                while (
                    st.av_next < KC
                    and (st.av_next, 0) in st.exp_of
                    and (st.av_next, 1) in st.exp_of
                ):
                    if upto is not None and st.av_next >= upto:
                        break
                    if cap is not None and n >= cap:
                        break
                    kc = st.av_next
                    for h_idx in range(2):
                        ex, p = st.exp_of.pop((kc, h_idx))
                        nc.tensor.matmul(
                            st.pavs[h_idx][0 : D + 1, :],
                            v_sb[:, kc, 2 * pair + h_idx, :],
                            ex[:, p, :],
                            start=(kc == 0),
                            stop=(kc == KC - 1),
                        )
                    st.av_next += 1
                    n += 1

            def run_stream(
                st, pair, qb, kcs, extras=None, av_start=0, av_cap=None,
                av_upto=None, flush_end=False,
            ):
                extras = dict(extras or {})
                for kc in kcs:
                    fl0 = st.fl
                    emit_slice(st, pair, qb, kc, 0)
                    emit_slice(st, pair, qb, kc, 1)
                    if st.full is not None:
                        do_flush(st)
                    if st.fl > fl0:
                        if st.fl in extras:
                            extras.pop(st.fl)()
                        if st.fl >= av_start:
                            cap = (
                                av_cap[st.fl % 2]
                                if isinstance(av_cap, tuple)
                                else av_cap
                            )
                            av_drain(st, pair, cap=cap, upto=av_upto)
                for k in sorted(extras):
                    extras.pop(k)()
                if flush_end and st.group is not None:
                    # never leave a partial group holding a ps3 ring slot
                    # across other ps3 users (prefix transposes/kT/v)
                    st.full = (st.group[0], st.group[1], st.group[2])
                    st.group = None
                    do_flush(st)
                    if st.fl >= av_start:
                        cap = (
                            av_cap[st.fl % 2] if isinstance(av_cap, tuple) else av_cap
                        )
                        av_drain(st, pair, cap=cap, upto=av_upto)

            def close_stream(st, pair):
                if st.group is not None:
                    st.full = (st.group[0], st.group[1], st.group[2])
                    st.group = None
                if st.full is not None:
                    do_flush(st)
                av_drain(st, pair)
                assert st.av_next == KC, st.av_next
                vts = []
                for h_idx in range(2):
                    vt = valT_pool.tile([D + 1, 512], f32, tag="valT", bufs=4)
                    nc.vector.tensor_copy(out=vt, in_=st.pavs[h_idx][0 : D + 1, :])
                    vts.append(vt)
                st.pavs = None
                return vts

            def transpose_half(vts, qb, pair, half, on_pav=False):
                h = 2 * pair + half
                if on_pav:
                    bank = psum_av.tile([128, 512], f32, tag="pav", name="tsc")
                else:
                    pt = ps3.tile([128, 3, 512], f32, tag="ps3")
                    bank = pt[:, 0, :]
                pv = bank.rearrange("p (j d) -> p j d", d=128)
                for j in range(4):
                    nc.tensor.transpose(
                        pv[:, j, 0 : D + 1],
                        vts[half][:, j * 128 : (j + 1) * 128],
                        ident[0 : D + 1, 0 : D + 1],
                    )
                nc.vector.tensor_copy(
                    out=val_asm[:, qb * 4 : qb * 4 + 4, h, :],
                    in_=pv[:, :, 0 : D + 1],
                )

            # ---- phase 5 per qb half ----
            class Ph5:
                def __init__(self, qtiles):
                    self.qtiles = qtiles
                    self.ots = []
                    self.m4 = stats.tile([128, 4], f32, tag="m4b", name="m4")
                    self.a4b = stats.tile([128, 4], f32, tag="a4b", name="a4b")
                    self.r4 = stats.tile([128, 4], f32, tag="r4b", name="r4")

                def stats_one(self, jj, eng=None, mul_act=False):
                    eng = eng or nc.vector
                    qtile = self.qtiles[jj]
                    va = val_asm[:, qtile]
                    ot = out_pool.tile([128, C], f32, tag="ot", name="ot")
                    rs8 = stats.tile([128, 8], f32, tag="rs8", name="rs8")
                    nc.vector.reciprocal(
                        out=rs8,
                        in_=va[:, :, D : D + 1].rearrange("p h one -> p (h one)"),
                    )
                    for h in range(H):
                        if mul_act:
                            nc.scalar.activation(
                                out=ot[:, h * D : (h + 1) * D],
                                in_=va[:, h, 0:D],
                                func=AF.Identity,
                                scale=rs8[:, h : h + 1],
                            )
                        else:
                            eng.tensor_scalar_mul(
                                out=ot[:, h * D : (h + 1) * D],
                                in0=va[:, h, 0:D],
                                scalar1=rs8[:, h : h + 1],
                            )
                    if use_b_v:
                        eng.tensor_add(out=ot, in0=ot, in1=bv_t)
                    st6 = stats.tile([128, 6], f32, tag="bn6", name="st6")
                    nc.vector.bn_stats(out=st6, in_=ot)
                    mv = stats.tile([128, 2], f32, tag="mv", name="mv")
                    nc.vector.bn_aggr(out=mv, in_=st6)
                    nc.vector.tensor_copy(out=self.m4[:, jj : jj + 1], in_=mv[:, 0:1])
                    nc.vector.tensor_copy(out=self.a4b[:, jj : jj + 1], in_=mv[:, 1:2])
                    self.ots.append(ot)

                def rsqrt(self):
                    nc.vector.tensor_scalar_add(out=self.a4b, in0=self.a4b, scalar1=EPS)
                    rsqrt_into(self.r4, self.a4b, 4, "p5")
                    self.nmr = stats.tile([128, 4], f32, tag="nmr", name="nmr")
                    nc.vector.tensor_mul(out=self.nmr, in0=self.m4, in1=self.r4)
                    nc.vector.tensor_scalar_mul(out=self.nmr, in0=self.nmr, scalar1=-1.0)

                def fin_one(self, jj, eng=None, use_act=False):
                    eng = eng or nc.vector
                    qtile = self.qtiles[jj]
                    ot = self.ots[jj]
                    if use_act:
                        # (x - m)*r on ScalarE (idle at the tail): x*r - m*r
                        nc.scalar.activation(
                            out=ot,
                            in_=ot,
                            func=AF.Identity,
                            scale=self.r4[:, jj : jj + 1],
                            bias=self.nmr[:, jj : jj + 1],
                        )
                    else:
                        eng.tensor_scalar(
                            out=ot,
                            in0=ot,
                            scalar1=self.m4[:, jj : jj + 1],
                            scalar2=self.r4[:, jj : jj + 1],
                            op0=ALU.subtract,
                            op1=ALU.mult,
                        )
                    if use_g_post:
                        eng.tensor_mul(out=ot, in0=ot, in1=g_post_t)
                    if use_beta_post:
                        eng.tensor_add(out=ot, in0=ot, in1=beta_post_t)
                    eng.tensor_add(out=ot, in0=ot, in1=xn_rows[:, qtile, :])
                    nc.sync.dma_start(
                        out=out[qtile * 128 : (qtile + 1) * 128, :], in_=ot
                    )

            # ================= emission =================
            states = [[AttState() for _ in range(NQB)] for _ in range(NPAIR)]
            st00 = states[0][0]

            # phase 1+2: LN -> xnT transposes -> kT0/qT0 -> v (lagged),
            # with the pair0-qb0 score/exp/AV stream trickling one rc behind
            for rc in range(NT // 4):
                if rc >= 1:
                    run_stream(
                        st00, 0, 0, range(4 * (rc - 1), 4 * rc),
                        av_upto=4 * (rc - 1), flush_end=True,
                    )
                    # v for kc [4(rc-1), 4rc): xnT inputs are from iteration
                    # rc-1, so these MMs fill the PE idle window while this
                    # iteration's LN chain runs on DVE (keeps HAM warm)
                    for kc in range(4 * (rc - 1), 4 * rc):
                        produce_v(kc)
                xts, mvs = [], []
                for j in range(4):
                    i = 4 * rc + j
                    xt = ln_in.tile([128, C], f32, tag="xt")
                    nc.sync.dma_start(out=xt, in_=xb[i * 128 : (i + 1) * 128, :])
                    st6 = stats.tile([128, 6], f32, tag="bn6")
                    nc.vector.bn_stats(out=st6, in_=xt)
                    mv = stats.tile([128, 2], f32, tag="mv")
                    nc.vector.bn_aggr(out=mv, in_=st6)
                    xts.append(xt)
                    mvs.append(mv)
                a4 = stats.tile([128, 4], f32, tag="a4")
                for j in range(4):
                    nc.vector.tensor_copy(out=a4[:, j : j + 1], in_=mvs[j][:, 1:2])
                nc.vector.tensor_scalar_add(out=a4, in0=a4, scalar1=EPS)
                r4 = stats.tile([128, 4], f32, tag="r4")
                rsqrt_into(r4, a4, 4, "p1")
                for j in range(4):
                    i = 4 * rc + j
                    eng = nc.vector
                    xbf = ln_in.tile([128, C], bf16, tag="xbf", bufs=4)
                    if i < QT or use_g_pre or use_beta_pre:
                        dst = xn_rows[:, i, :] if i < QT else xts[j]
                        eng.tensor_scalar(
                            out=dst,
                            in0=xts[j],
                            scalar1=mvs[j][:, 0:1],
                            scalar2=r4[:, j : j + 1],
                            op0=ALU.subtract,
                            op1=ALU.mult,
                        )
                        if use_g_pre:
                            eng.tensor_mul(out=dst, in0=dst, in1=g_pre_t)
                        if use_beta_pre:
                            eng.tensor_add(out=dst, in0=dst, in1=beta_pre_t)
                        eng.tensor_copy(out=xbf, in_=dst)
                    else:
                        eng.tensor_scalar(
                            out=xbf,
                            in0=xts[j],
                            scalar1=mvs[j][:, 0:1],
                            scalar2=r4[:, j : j + 1],
                            op0=ALU.subtract,
                            op1=ALU.mult,
                        )
                    transpose_into(xbf, i * 128)
                produce_kT(0, rc)
                if rc < NQB:
                    produce_qT(0, rc)
                # late prefix: next pair's kT ch0-3 on the ring
                if rc >= 6:
                    for r2 in range(2 * (rc - 6), 2 * (rc - 6) + 2):
                        produce_kT(1, r2)
                if rc == 0:
                    # bulk weight DMAs, after the first x tiles are queued
                    nc.sync.dma_start(out=w_v, in_=wv_d)
                    nc.sync.dma_start(out=w_k1, in_=wk1_d)
                    nc.sync.dma_start(out=w_q1, in_=wq1_d)
                    if use_b_v:
                        bv_t = consts.tile([128, C], f32)
                        nc.sync.dma_start(
                            out=bv_t, in_=bcast(bqkv[2 * C : 3 * C], 128)
                        )
                    if use_g_post:
                        g_post_t = consts.tile([128, C], f32)
                        nc.sync.dma_start(out=g_post_t, in_=bcast(g_post, 128))
                    if use_beta_post:
                        beta_post_t = consts.tile([128, C], f32)
                        nc.sync.dma_start(
                            out=beta_post_t, in_=bcast(beta_post, 128)
                        )
                if 2 <= rc <= 4:
                    # qT pair rc-1, both chunks in ONE ring slot
                    pq2 = ps3.tile([128, 3, 512], f32, tag="ps3", name="pq2")
                    for i in range(2):
                        for cc in range(CCH):
                            nc.tensor.matmul(
                                pq2[:, i, :],
                                wq(rc - 1, cc),
                                xnT[:, cc, i * 512 : (i + 1) * 512],
                                start=(cc == 0),
                                stop=(cc == CCH - 1),
                            )
                    for i in range(2):
                        if use_b_q:
                            nc.vector.tensor_scalar_add(
                                out=qTs[rc - 1][:, i * 512 : (i + 1) * 512],
                                in0=pq2[:, i, :],
                                scalar1=bq_t[:, rc - 1, :],
                            )
                        else:
                            nc.vector.tensor_copy(
                                out=qTs[rc - 1][:, i * 512 : (i + 1) * 512],
                                in_=pq2[:, i, :],
                            )

            # stream0 tail: last v chunks, remaining scores, full AV drain
            for kc in range(28, 32):
                produce_v(kc, eng=nc.vector)
            run_stream(st00, 0, 0, range(28, 32))
            vts_prev = close_stream(st00, 0)
            tp_prev = (0, 0)

            # streams 1..7, qb-interleaved: (p0,q1),(p1,q0),(p1,q1),...
            order = [(0, 1), (1, 0), (1, 1), (2, 0), (2, 1), (3, 0), (3, 1)]
            ph5a = Ph5([0, 1, 2, 3])
            for pair, qb in order:
                st = states[pair][qb]
                pp, pqb = tp_prev
                vp = vts_prev
                ex = {
                    1: (lambda vp=vp, pqb=pqb, pp=pp: transpose_half(vp, pqb, pp, 0, on_pav=True)),
                    2: (lambda vp=vp, pqb=pqb, pp=pp: transpose_half(vp, pqb, pp, 1, on_pav=True)),
                }
                av_start = 6
                if qb == 0 and pair + 1 < NPAIR:
                    for t in range(4):
                        ex[3 + t] = (
                            lambda pair=pair, t=t: produce_kT(pair + 1, t, on_pav=True)
                        )
                elif qb == 1 and pair + 1 < NPAIR:
                    for t in range(4):
                        ex[3 + t] = (
                            lambda pair=pair, t=t: produce_kT(
                                pair + 1, 4 + t, on_pav=True
                            )
                        )
                if (pair, qb) == (3, 1):
                    # phase5 for the qb0 rows rides the last stream
                    ex.update(
                        {
                            4: lambda: ph5a.stats_one(0),
                            6: lambda: ph5a.stats_one(1),
                            8: lambda: ph5a.stats_one(2),
                            10: lambda: ph5a.stats_one(3),
                            12: lambda: ph5a.rsqrt(),
                            14: lambda: ph5a.fin_one(0),
                            16: lambda: ph5a.fin_one(1),
                            18: lambda: ph5a.fin_one(2),
                            20: lambda: ph5a.fin_one(3),
                        }
                    )
                run_stream(
                    st, pair, qb, range(KC),
                    extras=ex, av_start=av_start, av_cap=(5, 4),
                )
                vts_prev = close_stream(st, pair)
                tp_prev = (pair, qb)

            # tail: last vt transposes + phase5 for qb1, DVE/Pool pipelined
            transpose_half(vts_prev, tp_prev[1], tp_prev[0], 0, on_pav=True)
            transpose_half(vts_prev, tp_prev[1], tp_prev[0], 1, on_pav=True)
            ph5b = Ph5([4, 5, 6, 7])
            ph5b.stats_one(0, mul_act=True)
            ph5b.stats_one(1)
            ph5b.stats_one(2, mul_act=True)
            ph5b.stats_one(3)
            ph5b.rsqrt()
            ph5b.fin_one(0, use_act=True)
            ph5b.fin_one(1)
            ph5b.fin_one(2, use_act=True)
            ph5b.fin_one(3)

    nc.compile()
    return nc


def kernel(x, w_qkv, b_qkv, g_pre, beta_pre, g_post, beta_post):
    import ml_dtypes
    from concourse.bass_utils import run_bass_kernel_spmd

    x = np.asarray(x, dtype=np.float32)
    w_qkv = np.asarray(w_qkv, dtype=np.float32)
    b_qkv = np.asarray(b_qkv, dtype=np.float32)
    g_pre = np.asarray(g_pre, dtype=np.float32)
    beta_pre = np.asarray(beta_pre, dtype=np.float32)
    g_post = np.asarray(g_post, dtype=np.float32)
    beta_post = np.asarray(beta_post, dtype=np.float32)

    flags = (
        not np.all(g_pre == 1.0),
        not np.all(beta_pre == 0.0),
        not np.all(g_post == 1.0),
        not np.all(beta_post == 0.0),
        not np.all(b_qkv[0:C] == 0.0),
        not np.all(b_qkv[2 * C : 3 * C] == 0.0),
    )
    # NOTE: b_qkv[C:2C] (the K bias) provably cancels in softmax and is
    # intentionally never applied.
    if flags not in _CACHE:
        _CACHE[flags] = _build(flags)
    nc = _CACHE[flags]

    w_bf = w_qkv.astype(ml_dtypes.bfloat16)
    wt = w_bf.reshape(4, 128, 3 * C)  # (cc, p, m)

    def blk(lo, hi):
        return np.ascontiguousarray(wt[:, :, lo:hi].transpose(1, 0, 2))

    wk0 = blk(C, C + 128)
    wq0 = blk(0, 128)
    wv = blk(2 * C, 3 * C)
    wk1 = blk(C + 128, 2 * C)
    wq1 = blk(128, C)
    in_maps = []
    for c in range(8):
        b = c // 4
        r = c % 4
        xrot = np.ascontiguousarray(
            np.concatenate([x[b, r * QR :], x[b, : r * QR]], axis=0)
        )
        in_maps.append(
            {
                "xb": xrot,
                "wk0": wk0,
                "wq0": wq0,
                "wv": wv,
                "wk1": wk1,
                "wq1": wq1,
                "b_qkv": b_qkv,
                "g_pre": g_pre,
                "beta_pre": beta_pre,
                "g_post": g_post,
                "beta_post": beta_post,
            }
        )

    global _last_in_maps
    _last_in_maps = in_maps
    res = run_bass_kernel_spmd(nc, in_maps, core_ids=list(range(8)))
    out = np.empty((B, N, C), dtype=np.float32)
    for c in range(8):
        b = c // 4
        r = c % 4
        out[b, r * QR : (r + 1) * QR] = res.results[c]["out"]
    return out


# revision 20
# speedup vs baseline: 1.0864x; 1.0060x over previous
"""Fused pre-LN + QKV + attention + post-LN + residual kernel for TRN2.

Problem (nn_Attention_86517821210894):
    B=2, N=4096, C=512, H=8, D=64
    xn  = LN(x) ; qkv = xn @ w_qkv + b ; per-(b,h) softmax attention
    val = LN(attn_out) ; out = xn + val

Sharding (8 cores, zero collectives):
    core c -> batch b = c // 4, query-row block r = c % 4 (1024 rows).
    Each core receives x[b] ROTATED so its query block is rows 0:1024
    (softmax and the value sum are permutation-invariant over keys),
    builds K/V for all 4096 keys, and produces out[b, r*1024:(r+1)*1024].

Design notes (v2, restructured from the 428us baseline using its trace):
  - ScalarE exp stream is the hard floor: 171 flushes x ~1.53us = 262us.
    Everything else is scheduled to keep that stream dense.
  - Score matmuls for the two heads of a pair are emitted ADJACENTLY so
    the PE runs them concurrently in the 64-row tiles (measured: a pair
    costs 512 cycles total, same as one solo MM).  The exp of a full
    PSUM group is emitted lazily - just before the next group alloc -
    so it never splits a pair.
  - w_qkv is DMA'd in 5 column-block pieces (K-pair0 and Q-pair0 first)
    so the first kT matmul no longer waits 14us for the full weight load.
  - v/kT/qT production is spread: kT(p+1) chunks ride the free pav PSUM
    banks during the first flushes of each stream (before AV claims
    them, av_start_flush=7/8); v rides the prefix ps3 ring just-in-time
    for the trailing AV.
  - Stream order interleaves qb: (p0,q0=prefix),(p0,q1),(p1,q0),... so
    each pair's kT production is split across the two preceding streams.
  - Tail: phase-5 post-LN work is split across DVE and GpSimd (Pool) and
    pipelined per 128-row tile to shrink the after-last-exp tail.
"""

import sys

sys.path.insert(0, "/opt/trn_rl_repo")

import numpy as np

B, N, C, H = 2, 4096, 512, 8
D = C // H
QR = N // 4  # query rows per core
EPS = 1e-5
SCALE = float(D) ** -0.5

_CACHE = {}


def _build(flags):
    (use_g_pre, use_beta_pre, use_g_post, use_beta_post, use_b_q, use_b_v) = flags

    import concourse.bacc as bacc
    import concourse.bass as bass
    import concourse.tile as tile
    from concourse import mybir
    from concourse.masks import make_identity

    f32 = mybir.dt.float32
    bf16 = mybir.dt.bfloat16
    AF = mybir.ActivationFunctionType
    ALU = mybir.AluOpType

    nc = bacc.Bacc(
        "TRN2", target_bir_lowering=False, debug=False, enable_asserts=False
    )

    xb = nc.dram_tensor("xb", [N, C], f32, kind="ExternalInput").ap()
    wk0_d = nc.dram_tensor("wk0", [128, C // 128, 128], bf16, kind="ExternalInput").ap()
    wq0_d = nc.dram_tensor("wq0", [128, C // 128, 128], bf16, kind="ExternalInput").ap()
    wv_d = nc.dram_tensor("wv", [128, C // 128, C], bf16, kind="ExternalInput").ap()
    wk1_d = nc.dram_tensor("wk1", [128, C // 128, 384], bf16, kind="ExternalInput").ap()
    wq1_d = nc.dram_tensor("wq1", [128, C // 128, 384], bf16, kind="ExternalInput").ap()
    bqkv = nc.dram_tensor("b_qkv", [3 * C], f32, kind="ExternalInput").ap()
    g_pre = nc.dram_tensor("g_pre", [C], f32, kind="ExternalInput").ap()
    beta_pre = nc.dram_tensor("beta_pre", [C], f32, kind="ExternalInput").ap()
    g_post = nc.dram_tensor("g_post", [C], f32, kind="ExternalInput").ap()
    beta_post = nc.dram_tensor("beta_post", [C], f32, kind="ExternalInput").ap()
    out = nc.dram_tensor("out", [QR, C], f32, kind="ExternalOutput").ap()

    NT = N // 128  # 32 row tiles of x[b]
    QT = QR // 128  # 8 row tiles of the query block
    CCH = C // 128  # 4 contraction chunks
    KC = N // 128  # 32 key chunks
    NPAIR = H // 2
    NQB = QR // 512  # 2 query blocks of 512

    def bcast(vec_ap, p):
        return bass.AP(
            tensor=vec_ap.tensor, offset=vec_ap.offset, ap=[[0, p], *vec_ap.ap]
        )

    with tile.TileContext(nc) as tc:
        with (
            tc.tile_pool(name="consts", bufs=1) as consts,
            tc.tile_pool(name="ln_in", bufs=6) as ln_in,
            tc.tile_pool(name="stats", bufs=8) as stats,
            tc.tile_pool(name="xnrow", bufs=1) as xnrow_pool,
            tc.tile_pool(name="xnT", bufs=1) as xnT_pool,
            tc.tile_pool(name="vsb", bufs=1) as v_pool,
            tc.tile_pool(name="kT", bufs=1) as kT_pool,
            tc.tile_pool(name="qT", bufs=1) as qT_pool,
            tc.tile_pool(name="expT", bufs=7) as expT_pool,
            tc.tile_pool(name="valT", bufs=2) as valT_pool,
            tc.tile_pool(name="valasm", bufs=1) as val_pool,
            tc.tile_pool(name="outp", bufs=4) as out_pool,
            tc.tile_pool(name="ps3", bufs=2, space="PSUM") as ps3,
            tc.tile_pool(name="pav", bufs=2, space="PSUM") as psum_av,
        ):
            # ---- warmup burst: bring the PE HAM to K=8/8 immediately ----
            dummy = consts.tile([128, 512], bf16)
            nc.vector.memset(dummy, 0.0)
            pw = ps3.tile([128, 3, 512], f32, tag="ps3")
            for _ in range(20):
                nc.tensor.matmul(pw[:, 0, :], dummy[:, 0:128], dummy)
            del pw

            # ---- constants ----
            ident = consts.tile([128, 128], f32)
            make_identity(nc, ident)
            ident_bf = consts.tile([128, 128], bf16)
            make_identity(nc, ident_bf)
            eps_t = consts.tile([128, 1], f32)
            nc.vector.memset(eps_t, EPS)
            seed_b = consts.tile([128, 1], f32)
            nc.vector.memset(seed_b, 0.5 * 0.6931471805599453 * 127.0)

            # ---- weights: host-prearranged blocks, contiguous DMAs ----
            w_k0 = consts.tile([128, CCH, 128], bf16)
            nc.sync.dma_start(out=w_k0, in_=wk0_d)
            w_q0 = consts.tile([128, CCH, 128], bf16)
            nc.sync.dma_start(out=w_q0, in_=wq0_d)
            w_v = consts.tile([128, CCH, C], bf16)
            w_k1 = consts.tile([128, CCH, 384], bf16)
            w_q1 = consts.tile([128, CCH, 384], bf16)

            g_pre_t = beta_pre_t = g_post_t = beta_post_t = None
            if use_g_pre:
                g_pre_t = consts.tile([128, C], f32)
                nc.sync.dma_start(out=g_pre_t, in_=bcast(g_pre, 128))
            if use_beta_pre:
                beta_pre_t = consts.tile([128, C], f32)
                nc.sync.dma_start(out=beta_pre_t, in_=bcast(beta_pre, 128))
            bq_t = None
            if use_b_q:
                bq_t = consts.tile([128, CCH, 1], f32)
                nc.sync.dma_start(
                    out=bq_t, in_=bqkv[0:C].rearrange("(cc p) -> p cc 1", p=128)
                )
            g_post_t = beta_post_t = bv_t = None

            def wk(pair, cc):
                if pair == 0:
                    return w_k0[:, cc, :]
                return w_k1[:, cc, (pair - 1) * 128 : pair * 128]

            def wq(pair, cc):
                if pair == 0:
                    return w_q0[:, cc, :]
                return w_q1[:, cc, (pair - 1) * 128 : pair * 128]

            # ---- persistent tensors ----
            xn_rows = xnrow_pool.tile([128, QT, C], f32)
            xnT = xnT_pool.tile([128, CCH, N], bf16)
            v_sb = v_pool.tile([128, KC, H, D + 1], bf16)
            val_asm = val_pool.tile([128, QT, H, D + 1], f32)
            kTs = [
                kT_pool.tile([128, N], bf16, tag="kT", bufs=4, name=f"kT{i}")
                for i in range(4)
            ]
            qTs = [
                qT_pool.tile([128, QR], bf16, tag="qT", bufs=4, name=f"qT{i}")
                for i in range(4)
            ]

            nc.vector.memset(v_sb[:, :, :, D : D + 1], 1.0)

            def rsqrt_into(dst, a4, w, tag):
                """dst = 1/sqrt(a4), a4 > 0, [128, w] f32 (Exp-seed + Newton)."""
                ai = a4.bitcast(mybir.dt.int32)
                fi = stats.tile([128, w], f32, tag=tag + "_f")
                nc.vector.tensor_copy(out=fi, in_=ai)
                nc.scalar.activation(
                    out=dst,
                    in_=fi,
                    func=AF.Exp,
                    scale=-0.5 * 0.6931471805599453 / 8388608.0,
                    bias=seed_b,
                )
                for _ in range(1):
                    t = stats.tile([128, w], f32, tag=tag + "_t")
                    nc.vector.tensor_mul(out=t, in0=dst, in1=dst)
                    nc.vector.tensor_mul(out=t, in0=t, in1=a4)
                    nc.vector.tensor_scalar(
                        out=t,
                        in0=t,
                        scalar1=-0.5,
                        scalar2=1.5,
                        op0=ALU.mult,
                        op1=ALU.add,
                    )
                    nc.vector.tensor_mul(out=dst, in0=dst, in1=t)

            def transpose_into(src, col0):
                ps = ps3.tile([128, 3, 1024], bf16, tag="ps3")
                pview = ps[:, 0, 0:512].rearrange("p (c n) -> p c n", n=128)
                for cc in range(CCH):
                    nc.tensor.transpose(
                        pview[:, cc, :], src[:, cc * 128 : (cc + 1) * 128], ident_bf
                    )
                nc.vector.tensor_copy(
                    out=xnT[:, :, col0 : col0 + 128], in_=pview
                )

            def produce_v(kc, eng=None):
                pv = ps3.tile([128, 3, 512], f32, tag="ps3")
                for cc in range(CCH):
                    nc.tensor.matmul(
                        pv[:, 0, :],
                        xnT[:, cc, kc * 128 : (kc + 1) * 128],
                        w_v[:, cc, :],
                        start=(cc == 0),
                        stop=(cc == CCH - 1),
                    )
                src = pv[:, 0, :].rearrange("p (h d) -> p h d", d=D)
                dst = v_sb[:, kc, :, 0:D]
                if use_b_v:
                    nc.vector.tensor_add(
                        out=dst, in0=src, in1=bv_t.rearrange("p (h d) -> p h d", d=D)
                    )
                elif eng is None:
                    nc.scalar.copy(out=dst, in_=src)
                else:
                    eng.tensor_copy(out=dst, in_=src)

            def produce_kT(pair, rc, on_pav=False, evac=None):
                kT = kTs[pair]
                if on_pav:
                    pk = psum_av.tile([128, 512], f32, tag="pav", name="pkv")
                else:
                    pk3 = ps3.tile([128, 3, 512], f32, tag="ps3", name="pk3")
                    pk = pk3[:, 0, :]
                for cc in range(CCH):
                    nc.tensor.matmul(
                        pk,
                        wk(pair, cc),
                        xnT[:, cc, rc * 512 : (rc + 1) * 512],
                        start=(cc == 0),
                        stop=(cc == CCH - 1),
                    )
                if evac is None:
                    nc.vector.tensor_copy(
                        out=kT[:, rc * 512 : (rc + 1) * 512], in_=pk
                    )
                else:
                    evac.copy(out=kT[:, rc * 512 : (rc + 1) * 512], in_=pk)

            def produce_qT(pair, rc, on_pav=False):
                qT = qTs[pair]
                if on_pav:
                    pq = psum_av.tile([128, 512], f32, tag="pav", name="pqv")
                else:
                    pq3 = ps3.tile([128, 3, 512], f32, tag="ps3", name="pq3")
                    pq = pq3[:, 0, :]
                for cc in range(CCH):
                    nc.tensor.matmul(
                        pq,
                        wq(pair, cc),
                        xnT[:, cc, rc * 512 : (rc + 1) * 512],
                        start=(cc == 0),
                        stop=(cc == CCH - 1),
                    )
                if use_b_q:
                    nc.vector.tensor_scalar_add(
                        out=qT[:, rc * 512 : (rc + 1) * 512],
                        in0=pq,
                        scalar1=bq_t[:, pair, :],
                    )
                else:
                    nc.vector.tensor_copy(
                        out=qT[:, rc * 512 : (rc + 1) * 512], in_=pq
                    )

            # ---- attention stream machinery ----
            class AttState:
                def __init__(self):
                    self.group = None  # (tile, pos, pending)
                    self.full = None
                    self.exp_of = {}
                    self.pavs = None
                    self.av_next = 0
                    self.fl = 0

            def do_flush(st):
                tile_, pos, pending = st.full
                ex = expT_pool.tile([128, 3, 512], bf16, tag="expT")
                nc.scalar.activation(
                    out=ex[:, 0:pos, :],
                    in_=tile_[:, 0:pos, :],
                    func=AF.Exp,
                    scale=SCALE,
                )
                for key, p in pending:
                    st.exp_of[key] = (ex, p)
                st.full = None
                st.fl += 1

            def emit_slice(st, pair, qb, kc, h_idx):
                if st.group is None:
                    if st.full is not None:
                        do_flush(st)
                    st.group = [
                        ps3.tile([128, 3, 512], f32, tag="ps3", name="grp"),
                        0,
                        [],
                    ]
                g = st.group
                base = h_idx * 64
                kT, qT = kTs[pair], qTs[pair]
                nc.tensor.matmul(
                    g[0][:, g[1], :],
                    kT[base : base + 64, kc * 128 : (kc + 1) * 128],
                    qT[base : base + 64, qb * 512 : (qb + 1) * 512],
                )
                g[2].append(((kc, h_idx), g[1]))
                g[1] += 1
                if g[1] == 3:
                    st.full = (g[0], 3, g[2])
                    st.group = None

            def av_drain(st, pair, cap=None, upto=None):
                if st.pavs is None:
                    pav_lo = psum_av.tile([128, 512], f32, tag="pav")
                    pav_hi = psum_av.tile([128, 512], f32, tag="pav")
                    st.pavs = (pav_lo, pav_hi)
                n = 0
                while (
                    st.av_next < KC
                    and (st.av_next, 0) in st.exp_of
                    and (st.av_next, 1) in st.exp_of
                ):
                    if upto is not None and st.av_next >= upto:
                        break
                    if cap is not None and n >= cap:
                        break
                    kc = st.av_next
                    for h_idx in range(2):
                        ex, p = st.exp_of.pop((kc, h_idx))
                        nc.tensor.matmul(
                            st.pavs[h_idx][0 : D + 1, :],
                            v_sb[:, kc, 2 * pair + h_idx, :],
                            ex[:, p, :],
                            start=(kc == 0),
                            stop=(kc == KC - 1),
                        )
                    st.av_next += 1
                    n += 1

            def run_stream(
                st, pair, qb, kcs, extras=None, av_start=0, av_cap=None,
                av_upto=None, flush_end=False,
            ):
                extras = dict(extras or {})
                for kc in kcs:
                    fl0 = st.fl
                    emit_slice(st, pair, qb, kc, 0)
                    emit_slice(st, pair, qb, kc, 1)
                    if st.full is not None:
                        do_flush(st)
                    if st.fl > fl0:
                        if st.fl in extras:
                            extras.pop(st.fl)()
                        if st.fl >= av_start:
                            cap = (
                                av_cap[st.fl % 2]
                                if isinstance(av_cap, tuple)
                                else av_cap
                            )
                            av_drain(st, pair, cap=cap, upto=av_upto)
                for k in sorted(extras):
                    extras.pop(k)()
                if flush_end and st.group is not None:
                    # never leave a partial group holding a ps3 ring slot
                    # across other ps3 users (prefix transposes/kT/v)
                    st.full = (st.group[0], st.group[1], st.group[2])
                    st.group = None
                    do_flush(st)
                    if st.fl >= av_start:
                        cap = (
                            av_cap[st.fl % 2] if isinstance(av_cap, tuple) else av_cap
                        )
                        av_drain(st, pair, cap=cap, upto=av_upto)

            def close_stream(st, pair):
                if st.group is not None:
                    st.full = (st.group[0], st.group[1], st.group[2])
                    st.group = None
                if st.full is not None:
                    do_flush(st)
                av_drain(st, pair)
                assert st.av_next == KC, st.av_next
                vts = []
                for h_idx in range(2):
                    vt = valT_pool.tile([D + 1, 512], f32, tag="valT", bufs=4)
                    nc.vector.tensor_copy(out=vt, in_=st.pavs[h_idx][0 : D + 1, :])
                    vts.append(vt)
                st.pavs = None
                return vts

            def transpose_half(vts, qb, pair, half, on_pav=False):
                h = 2 * pair + half
                if on_pav:
                    bank = psum_av.tile([128, 512], f32, tag="pav", name="tsc")
                else:
                    pt = ps3.tile([128, 3, 512], f32, tag="ps3")
                    bank = pt[:, 0, :]
                pv = bank.rearrange("p (j d) -> p j d", d=128)
                for j in range(4):
                    nc.tensor.transpose(
                        pv[:, j, 0 : D + 1],
                        vts[half][:, j * 128 : (j + 1) * 128],
                        ident[0 : D + 1, 0 : D + 1],
                    )
                nc.vector.tensor_copy(
                    out=val_asm[:, qb * 4 : qb * 4 + 4, h, :],
                    in_=pv[:, :, 0 : D + 1],
                )

            # ---- phase 5 per qb half ----
            class Ph5:
                def __init__(self, qtiles):
                    self.qtiles = qtiles
                    self.ots = []
                    self.parts = []
                    self.m4 = stats.tile([128, 4], f32, tag="m4b", name="m4")
                    self.a4b = stats.tile([128, 4], f32, tag="a4b", name="a4b")
                    self.r4 = stats.tile([128, 4], f32, tag="r4b", name="r4")

                def stats_one(self, jj, eng=None, mul_act=False):
                    self.early_one(jj, nheads=H, eng=eng, mul_act=mul_act)
                    self.late_one(jj, nheads=H, eng=eng)

                def early_one(self, jj, nheads=6, eng=None, mul_act=False):
                    # denominator muls + partial bn_stats for heads < nheads
                    eng = eng or nc.vector
                    qtile = self.qtiles[jj]
                    va = val_asm[:, qtile]
                    ot = out_pool.tile([128, C], f32, tag="ot", name="ot")
                    rs8 = stats.tile([128, 8], f32, tag="rs8", name="rs8", bufs=4)
                    nc.vector.reciprocal(
                        out=rs8[:, 0:nheads],
                        in_=va[:, 0:nheads, D : D + 1].rearrange(
                            "p h one -> p (h one)"
                        ),
                    )
                    for h in range(nheads):
                        if mul_act:
                            nc.scalar.activation(
                                out=ot[:, h * D : (h + 1) * D],
                                in_=va[:, h, 0:D],
                                func=AF.Identity,
                                scale=rs8[:, h : h + 1],
                            )
                        else:
                            eng.tensor_scalar_mul(
                                out=ot[:, h * D : (h + 1) * D],
                                in0=va[:, h, 0:D],
                                scalar1=rs8[:, h : h + 1],
                            )
                    if use_b_v:
                        eng.tensor_add(
                            out=ot[:, 0 : nheads * D],
                            in0=ot[:, 0 : nheads * D],
                            in1=bv_t[:, 0 : nheads * D],
                        )
                    st6ab = stats.tile([128, 2, 6], f32, tag="bn6ab", name="st6ab", bufs=4)
                    nc.vector.bn_stats(out=st6ab[:, 0, :], in_=ot[:, 0 : nheads * D])
                    self.ots.append(ot)
                    self.parts.append((rs8, st6ab, nheads))

                def late_one(self, jj, nheads=6, eng=None):
                    eng = eng or nc.vector
                    qtile = self.qtiles[jj]
                    va = val_asm[:, qtile]
                    ot = self.ots[jj]
                    rs8, st6ab, nh = self.parts[jj]
                    if nh < H:
                        nc.vector.reciprocal(
                            out=rs8[:, nh:H],
                            in_=va[:, nh:H, D : D + 1].rearrange(
                                "p h one -> p (h one)"
                            ),
                        )
                        for h in range(nh, H):
                            eng.tensor_scalar_mul(
                                out=ot[:, h * D : (h + 1) * D],
                                in0=va[:, h, 0:D],
                                scalar1=rs8[:, h : h + 1],
                            )
                        if use_b_v:
                            eng.tensor_add(
                                out=ot[:, nh * D : C],
                                in0=ot[:, nh * D : C],
                                in1=bv_t[:, nh * D : C],
                            )
                        nc.vector.bn_stats(
                            out=st6ab[:, 1, :], in_=ot[:, nh * D : C]
                        )
                        mv = stats.tile([128, 2], f32, tag="mv", name="mv")
                        nc.vector.bn_aggr(out=mv, in_=st6ab)
                    else:
                        mv = stats.tile([128, 2], f32, tag="mv", name="mv")
                        nc.vector.bn_aggr(out=mv, in_=st6ab[:, 0, :])
                    nc.vector.tensor_copy(out=self.m4[:, jj : jj + 1], in_=mv[:, 0:1])
                    nc.vector.tensor_copy(out=self.a4b[:, jj : jj + 1], in_=mv[:, 1:2])

                def rsqrt(self):
                    nc.vector.tensor_scalar_add(out=self.a4b, in0=self.a4b, scalar1=EPS)
                    rsqrt_into(self.r4, self.a4b, 4, "p5")
                    self.nmr = stats.tile([128, 4], f32, tag="nmr", name="nmr")
                    nc.vector.tensor_mul(out=self.nmr, in0=self.m4, in1=self.r4)
                    nc.vector.tensor_scalar_mul(out=self.nmr, in0=self.nmr, scalar1=-1.0)

                def fin_one(self, jj, eng=None, use_act=False):
                    eng = eng or nc.vector
                    qtile = self.qtiles[jj]
                    ot = self.ots[jj]
                    if use_act:
                        # (x - m)*r on ScalarE (idle at the tail): x*r - m*r
                        nc.scalar.activation(
                            out=ot,
                            in_=ot,
                            func=AF.Identity,
                            scale=self.r4[:, jj : jj + 1],
                            bias=self.nmr[:, jj : jj + 1],
                        )
                    else:
                        eng.tensor_scalar(
                            out=ot,
                            in0=ot,
                            scalar1=self.m4[:, jj : jj + 1],
                            scalar2=self.r4[:, jj : jj + 1],
                            op0=ALU.subtract,
                            op1=ALU.mult,
                        )
                    if use_g_post:
                        eng.tensor_mul(out=ot, in0=ot, in1=g_post_t)
                    if use_beta_post:
                        eng.tensor_add(out=ot, in0=ot, in1=beta_post_t)
                    eng.tensor_add(out=ot, in0=ot, in1=xn_rows[:, qtile, :])
                    nc.sync.dma_start(
                        out=out[qtile * 128 : (qtile + 1) * 128, :], in_=ot
                    )

            # ================= emission =================
            states = [[AttState() for _ in range(NQB)] for _ in range(NPAIR)]
            st00 = states[0][0]

            # phase 1+2: LN -> xnT transposes -> kT0/qT0 -> v (lagged),
            # with the pair0-qb0 score/exp/AV stream trickling one rc behind
            for rc in range(NT // 4):
                if rc >= 1:
                    run_stream(
                        st00, 0, 0, range(4 * (rc - 1), 4 * rc),
                        av_upto=4 * (rc - 1), flush_end=True,
                    )
                    # v for kc [4(rc-1), 4rc): xnT inputs are from iteration
                    # rc-1, so these MMs fill the PE idle window while this
                    # iteration's LN chain runs on DVE (keeps HAM warm)
                    for kc in range(4 * (rc - 1), 4 * rc):
                        produce_v(kc)
                xts, mvs = [], []
                for j in range(4):
                    i = 4 * rc + j
                    xt = ln_in.tile([128, C], f32, tag="xt")
                    nc.sync.dma_start(out=xt, in_=xb[i * 128 : (i + 1) * 128, :])
                    st6 = stats.tile([128, 6], f32, tag="bn6")
                    nc.vector.bn_stats(out=st6, in_=xt)
                    mv = stats.tile([128, 2], f32, tag="mv")
                    nc.vector.bn_aggr(out=mv, in_=st6)
                    xts.append(xt)
                    mvs.append(mv)
                a4 = stats.tile([128, 4], f32, tag="a4")
                for j in range(4):
                    nc.vector.tensor_copy(out=a4[:, j : j + 1], in_=mvs[j][:, 1:2])
                nc.vector.tensor_scalar_add(out=a4, in0=a4, scalar1=EPS)
                r4 = stats.tile([128, 4], f32, tag="r4")
                rsqrt_into(r4, a4, 4, "p1")
                for j in range(4):
                    i = 4 * rc + j
                    eng = nc.vector
                    xbf = ln_in.tile([128, C], bf16, tag="xbf", bufs=4)
                    if i < QT or use_g_pre or use_beta_pre:
                        dst = xn_rows[:, i, :] if i < QT else xts[j]
                        eng.tensor_scalar(
                            out=dst,
                            in0=xts[j],
                            scalar1=mvs[j][:, 0:1],
                            scalar2=r4[:, j : j + 1],
                            op0=ALU.subtract,
                            op1=ALU.mult,
                        )
                        if use_g_pre:
                            eng.tensor_mul(out=dst, in0=dst, in1=g_pre_t)
                        if use_beta_pre:
                            eng.tensor_add(out=dst, in0=dst, in1=beta_pre_t)
                        eng.tensor_copy(out=xbf, in_=dst)
                    else:
                        eng.tensor_scalar(
                            out=xbf,
                            in0=xts[j],
                            scalar1=mvs[j][:, 0:1],
                            scalar2=r4[:, j : j + 1],
                            op0=ALU.subtract,
                            op1=ALU.mult,
                        )
                    transpose_into(xbf, i * 128)
                produce_kT(0, rc)
                if rc < NQB:
                    produce_qT(0, rc)
                # late prefix: next pair's kT ch0-3 on the ring
                if rc >= 6:
                    for r2 in range(2 * (rc - 6), 2 * (rc - 6) + 2):
                        produce_kT(1, r2)
                if rc == 0:
                    # bulk weight DMAs, after the first x tiles are queued
                    nc.sync.dma_start(out=w_v, in_=wv_d)
                    nc.sync.dma_start(out=w_k1, in_=wk1_d)
                    nc.sync.dma_start(out=w_q1, in_=wq1_d)
                    if use_b_v:
                        bv_t = consts.tile([128, C], f32)
                        nc.sync.dma_start(
                            out=bv_t, in_=bcast(bqkv[2 * C : 3 * C], 128)
                        )
                    if use_g_post:
                        g_post_t = consts.tile([128, C], f32)
                        nc.sync.dma_start(out=g_post_t, in_=bcast(g_post, 128))
                    if use_beta_post:
                        beta_post_t = consts.tile([128, C], f32)
                        nc.sync.dma_start(
                            out=beta_post_t, in_=bcast(beta_post, 128)
                        )
                if 2 <= rc <= 4:
                    # qT pair rc-1, both chunks in ONE ring slot
                    pq2 = ps3.tile([128, 3, 512], f32, tag="ps3", name="pq2")
                    for i in range(2):
                        for cc in range(CCH):
                            nc.tensor.matmul(
                                pq2[:, i, :],
                                wq(rc - 1, cc),
                                xnT[:, cc, i * 512 : (i + 1) * 512],
                                start=(cc == 0),
                                stop=(cc == CCH - 1),
                            )
                    for i in range(2):
                        if use_b_q:
                            nc.vector.tensor_scalar_add(
                                out=qTs[rc - 1][:, i * 512 : (i + 1) * 512],
                                in0=pq2[:, i, :],
                                scalar1=bq_t[:, rc - 1, :],
                            )
                        else:
                            nc.vector.tensor_copy(
                                out=qTs[rc - 1][:, i * 512 : (i + 1) * 512],
                                in_=pq2[:, i, :],
                            )

            # stream0 tail: last v chunks, remaining scores, full AV drain
            for kc in range(28, 32):
                produce_v(kc, eng=nc.vector)
            run_stream(st00, 0, 0, range(28, 32))
            vts_prev = close_stream(st00, 0)
            tp_prev = (0, 0)

            # streams 1..7, qb-interleaved: (p0,q1),(p1,q0),(p1,q1),...
            order = [(0, 1), (1, 0), (1, 1), (2, 0), (2, 1), (3, 0), (3, 1)]
            ph5a = Ph5([0, 1, 2, 3])
            ph5b = Ph5([4, 5, 6, 7])
            for pair, qb in order:
                st = states[pair][qb]
                pp, pqb = tp_prev
                vp = vts_prev
                ex = {
                    1: (lambda vp=vp, pqb=pqb, pp=pp: transpose_half(vp, pqb, pp, 0, on_pav=True)),
                    2: (lambda vp=vp, pqb=pqb, pp=pp: transpose_half(vp, pqb, pp, 1, on_pav=True)),
                }
                av_start = 6
                if qb == 0 and pair + 1 < NPAIR:
                    for t in range(4):
                        ex[3 + t] = (
                            lambda pair=pair, t=t: produce_kT(pair + 1, t, on_pav=True)
                        )
                elif qb == 1 and pair + 1 < NPAIR:
                    for t in range(4):
                        ex[3 + t] = (
                            lambda pair=pair, t=t: produce_kT(
                                pair + 1, 4 + t, on_pav=True
                            )
                        )
                if (pair, qb) == (3, 1):
                    # phase5-qb0 + the qb1 head-0..5 muls/stats ride here
                    ex.update(
                        {
                            4: lambda: ph5a.stats_one(0),
                            6: lambda: ph5a.stats_one(1),
                            8: lambda: ph5a.stats_one(2),
                            10: lambda: ph5a.stats_one(3),
                            12: lambda: ph5a.rsqrt(),
                            14: lambda: ph5a.fin_one(0),
                            15: lambda: ph5b.early_one(0),
                            16: lambda: ph5a.fin_one(1),
                            17: lambda: ph5b.early_one(1),
                            18: lambda: ph5a.fin_one(2),
                            19: lambda: ph5b.early_one(2),
                            20: lambda: ph5a.fin_one(3),
                            21: lambda: ph5b.early_one(3),
                        }
                    )
                run_stream(
                    st, pair, qb, range(KC),
                    extras=ex, av_start=av_start, av_cap=(5, 4),
                )
                vts_prev = close_stream(st, pair)
                tp_prev = (pair, qb)

            # tail: last vt transposes + phase5 for qb1, DVE/Pool pipelined
            transpose_half(vts_prev, tp_prev[1], tp_prev[0], 0, on_pav=True)
            transpose_half(vts_prev, tp_prev[1], tp_prev[0], 1, on_pav=True)
            for jj in range(4):
                ph5b.late_one(jj)
            ph5b.rsqrt()
            ph5b.fin_one(0, use_act=True)
            ph5b.fin_one(1)
            ph5b.fin_one(2, use_act=True)
            ph5b.fin_one(3)

    nc.compile()
    return nc


def kernel(x, w_qkv, b_qkv, g_pre, beta_pre, g_post, beta_post):
    import ml_dtypes
    from concourse.bass_utils import run_bass_kernel_spmd

    x = np.asarray(x, dtype=np.float32)
    w_qkv = np.asarray(w_qkv, dtype=np.float32)
    b_qkv = np.asarray(b_qkv, dtype=np.float32)
    g_pre = np.asarray(g_pre, dtype=np.float32)
    beta_pre = np.asarray(beta_pre, dtype=np.float32)
    g_post = np.asarray(g_post, dtype=np.float32)
    beta_post = np.asarray(beta_post, dtype=np.float32)

    flags = (
        not np.all(g_pre == 1.0),
        not np.all(beta_pre == 0.0),
        not np.all(g_post == 1.0),
        not np.all(beta_post == 0.0),
        not np.all(b_qkv[0:C] == 0.0),
        not np.all(b_qkv[2 * C : 3 * C] == 0.0),
    )
    # NOTE: b_qkv[C:2C] (the K bias) provably cancels in softmax and is
    # intentionally never applied.
    if flags not in _CACHE:
        _CACHE[flags] = _build(flags)
    nc = _CACHE[flags]

    w_bf = w_qkv.astype(ml_dtypes.bfloat16)
    wt = w_bf.reshape(4, 128, 3 * C)  # (cc, p, m)

    def blk(lo, hi):
        return np.ascontiguousarray(wt[:, :, lo:hi].transpose(1, 0, 2))

    wk0 = blk(C, C + 128)
    wq0 = blk(0, 128)
    wv = blk(2 * C, 3 * C)
    wk1 = blk(C + 128, 2 * C)
    wq1 = blk(128, C)
    in_maps = []
    for c in range(8):
        b = c // 4
        r = c % 4
        xrot = np.ascontiguousarray(
            np.concatenate([x[b, r * QR :], x[b, : r * QR]], axis=0)
        )
        in_maps.append(
            {
                "xb": xrot,
                "wk0": wk0,
                "wq0": wq0,
                "wv": wv,
                "wk1": wk1,
                "wq1": wq1,
                "b_qkv": b_qkv,
                "g_pre": g_pre,
                "beta_pre": beta_pre,
                "g_post": g_post,
                "beta_post": beta_post,
            }
        )

    global _last_in_maps
    _last_in_maps = in_maps
    res = run_bass_kernel_spmd(nc, in_maps, core_ids=list(range(8)))
    out = np.empty((B, N, C), dtype=np.float32)
    for c in range(8):
        b = c // 4
        r = c % 4
        out[b, r * QR : (r + 1) * QR] = res.results[c]["out"]
    return out


# revision 23
# speedup vs baseline: 1.0983x; 1.0109x over previous
"""Fused pre-LN + QKV + attention + post-LN + residual kernel for TRN2.

Problem (nn_Attention_86517821210894):
    B=2, N=4096, C=512, H=8, D=64
    xn  = LN(x) ; qkv = xn @ w_qkv + b ; per-(b,h) softmax attention
    val = LN(attn_out) ; out = xn + val

Sharding (8 cores, zero collectives):
    core c -> batch b = c // 4, query-row block r = c % 4 (1024 rows).
    Each core receives x[b] ROTATED so its query block is rows 0:1024
    (softmax and the value sum are permutation-invariant over keys),
    builds K/V for all 4096 keys, and produces out[b, r*1024:(r+1)*1024].

Design notes (v2, restructured from the 428us baseline using its trace):
  - ScalarE exp stream is the hard floor: 171 flushes x ~1.53us = 262us.
    Everything else is scheduled to keep that stream dense.
  - Score matmuls for the two heads of a pair are emitted ADJACENTLY so
    the PE runs them concurrently in the 64-row tiles (measured: a pair
    costs 512 cycles total, same as one solo MM).  The exp of a full
    PSUM group is emitted lazily - just before the next group alloc -
    so it never splits a pair.
  - w_qkv is DMA'd in 5 column-block pieces (K-pair0 and Q-pair0 first)
    so the first kT matmul no longer waits 14us for the full weight load.
  - v/kT/qT production is spread: kT(p+1) chunks ride the free pav PSUM
    banks during the first flushes of each stream (before AV claims
    them, av_start_flush=7/8); v rides the prefix ps3 ring just-in-time
    for the trailing AV.
  - Stream order interleaves qb: (p0,q0=prefix),(p0,q1),(p1,q0),... so
    each pair's kT production is split across the two preceding streams.
  - Tail: phase-5 post-LN work is split across DVE and GpSimd (Pool) and
    pipelined per 128-row tile to shrink the after-last-exp tail.
"""

import sys

sys.path.insert(0, "/opt/trn_rl_repo")

import numpy as np

B, N, C, H = 2, 4096, 512, 8
D = C // H
QR = N // 4  # query rows per core
EPS = 1e-5
SCALE = float(D) ** -0.5

_CACHE = {}


def _build(flags):
    (use_g_pre, use_beta_pre, use_g_post, use_beta_post, use_b_q, use_b_v) = flags

    import concourse.bacc as bacc
    import concourse.bass as bass
    import concourse.tile as tile
    from concourse import mybir
    from concourse.masks import make_identity

    f32 = mybir.dt.float32
    bf16 = mybir.dt.bfloat16
    AF = mybir.ActivationFunctionType
    ALU = mybir.AluOpType

    nc = bacc.Bacc(
        "TRN2", target_bir_lowering=False, debug=False, enable_asserts=False
    )

    xb = nc.dram_tensor("xb", [N, C], f32, kind="ExternalInput").ap()
    wk0_d = nc.dram_tensor("wk0", [128, C // 128, 128], bf16, kind="ExternalInput").ap()
    wq0_d = nc.dram_tensor("wq0", [128, C // 128, 128], bf16, kind="ExternalInput").ap()
    wv_d = nc.dram_tensor("wv", [128, C // 128, C], bf16, kind="ExternalInput").ap()
    wk1_d = nc.dram_tensor("wk1", [128, C // 128, 384], bf16, kind="ExternalInput").ap()
    wq1_d = nc.dram_tensor("wq1", [128, C // 128, 384], bf16, kind="ExternalInput").ap()
    bqkv = nc.dram_tensor("b_qkv", [3 * C], f32, kind="ExternalInput").ap()
    g_pre = nc.dram_tensor("g_pre", [C], f32, kind="ExternalInput").ap()
    beta_pre = nc.dram_tensor("beta_pre", [C], f32, kind="ExternalInput").ap()
    g_post = nc.dram_tensor("g_post", [C], f32, kind="ExternalInput").ap()
    beta_post = nc.dram_tensor("beta_post", [C], f32, kind="ExternalInput").ap()
    out = nc.dram_tensor("out", [QR, C], f32, kind="ExternalOutput").ap()

    NT = N // 128  # 32 row tiles of x[b]
    QT = QR // 128  # 8 row tiles of the query block
    CCH = C // 128  # 4 contraction chunks
    KC = N // 128  # 32 key chunks
    NPAIR = H // 2
    NQB = QR // 512  # 2 query blocks of 512

    def bcast(vec_ap, p):
        return bass.AP(
            tensor=vec_ap.tensor, offset=vec_ap.offset, ap=[[0, p], *vec_ap.ap]
        )

    with tile.TileContext(nc) as tc:
        with (
            tc.tile_pool(name="consts", bufs=1) as consts,
            tc.tile_pool(name="ln_in", bufs=6) as ln_in,
            tc.tile_pool(name="stats", bufs=8) as stats,
            tc.tile_pool(name="xnrow", bufs=1) as xnrow_pool,
            tc.tile_pool(name="xnT", bufs=1) as xnT_pool,
            tc.tile_pool(name="vsb", bufs=1) as v_pool,
            tc.tile_pool(name="kT", bufs=1) as kT_pool,
            tc.tile_pool(name="qT", bufs=1) as qT_pool,
            tc.tile_pool(name="expT", bufs=7) as expT_pool,
            tc.tile_pool(name="valT", bufs=2) as valT_pool,
            tc.tile_pool(name="valasm", bufs=1) as val_pool,
            tc.tile_pool(name="outp", bufs=4) as out_pool,
            tc.tile_pool(name="ps3", bufs=2, space="PSUM") as ps3,
            tc.tile_pool(name="pav", bufs=2, space="PSUM") as psum_av,
        ):
            # ---- warmup burst: bring the PE HAM to K=8/8 immediately ----
            dummy = consts.tile([128, 512], bf16)
            nc.vector.memset(dummy, 0.0)
            pw = ps3.tile([128, 3, 512], f32, tag="ps3")
            for _ in range(10):
                nc.tensor.matmul(pw[:, 0, :], dummy[:, 0:128], dummy)
            del pw

            # ---- constants ----
            ident = consts.tile([128, 128], f32)
            make_identity(nc, ident)
            ident_bf = consts.tile([128, 128], bf16)
            make_identity(nc, ident_bf)
            eps_t = consts.tile([128, 1], f32)
            nc.vector.memset(eps_t, EPS)
            seed_b = consts.tile([128, 1], f32)
            nc.vector.memset(seed_b, 0.5 * 0.6931471805599453 * 127.0)

            # ---- weights: host-prearranged blocks, contiguous DMAs ----
            w_k0 = consts.tile([128, CCH, 128], bf16)
            nc.sync.dma_start(out=w_k0, in_=wk0_d)
            w_q0 = consts.tile([128, CCH, 128], bf16)
            nc.sync.dma_start(out=w_q0, in_=wq0_d)
            w_v = consts.tile([128, CCH, C], bf16)
            w_k1 = consts.tile([128, CCH, 384], bf16)
            w_q1 = consts.tile([128, CCH, 384], bf16)

            g_pre_t = beta_pre_t = g_post_t = beta_post_t = None
            if use_g_pre:
                g_pre_t = consts.tile([128, C], f32)
                nc.sync.dma_start(out=g_pre_t, in_=bcast(g_pre, 128))
            if use_beta_pre:
                beta_pre_t = consts.tile([128, C], f32)
                nc.sync.dma_start(out=beta_pre_t, in_=bcast(beta_pre, 128))
            bq_t = None
            if use_b_q:
                bq_t = consts.tile([128, CCH, 1], f32)
                nc.sync.dma_start(
                    out=bq_t, in_=bqkv[0:C].rearrange("(cc p) -> p cc 1", p=128)
                )
            g_post_t = beta_post_t = bv_t = None

            def wk(pair, cc):
                if pair == 0:
                    return w_k0[:, cc, :]
                return w_k1[:, cc, (pair - 1) * 128 : pair * 128]

            def wq(pair, cc):
                if pair == 0:
                    return w_q0[:, cc, :]
                return w_q1[:, cc, (pair - 1) * 128 : pair * 128]

            # ---- persistent tensors ----
            xn_rows = xnrow_pool.tile([128, QT, C], f32)
            xnT = xnT_pool.tile([128, CCH, N], bf16)
            v_sb = v_pool.tile([128, KC, H, D + 1], bf16)
            val_asm = val_pool.tile([128, QT, H, D + 1], f32)
            kTs = [
                kT_pool.tile([128, N], bf16, tag="kT", bufs=4, name=f"kT{i}")
                for i in range(4)
            ]
            qTs = [
                qT_pool.tile([128, QR], bf16, tag="qT", bufs=4, name=f"qT{i}")
                for i in range(4)
            ]

            nc.vector.memset(v_sb[:, :, :, D : D + 1], 1.0)

            def rsqrt_into(dst, a4, w, tag):
                """dst = 1/sqrt(a4), a4 > 0, [128, w] f32 (Exp-seed + Newton)."""
                ai = a4.bitcast(mybir.dt.int32)
                fi = stats.tile([128, w], f32, tag=tag + "_f")
                nc.vector.tensor_copy(out=fi, in_=ai)
                nc.scalar.activation(
                    out=dst,
                    in_=fi,
                    func=AF.Exp,
                    scale=-0.5 * 0.6931471805599453 / 8388608.0,
                    bias=seed_b,
                )
                for _ in range(1):
                    t = stats.tile([128, w], f32, tag=tag + "_t")
                    nc.vector.tensor_mul(out=t, in0=dst, in1=dst)
                    nc.vector.tensor_mul(out=t, in0=t, in1=a4)
                    nc.vector.tensor_scalar(
                        out=t,
                        in0=t,
                        scalar1=-0.5,
                        scalar2=1.5,
                        op0=ALU.mult,
                        op1=ALU.add,
                    )
                    nc.vector.tensor_mul(out=dst, in0=dst, in1=t)

            def transpose_into(src, col0):
                ps = ps3.tile([128, 3, 1024], bf16, tag="ps3")
                pview = ps[:, 0, 0:512].rearrange("p (c n) -> p c n", n=128)
                for cc in range(CCH):
                    nc.tensor.transpose(
                        pview[:, cc, :], src[:, cc * 128 : (cc + 1) * 128], ident_bf
                    )
                nc.vector.tensor_copy(
                    out=xnT[:, 0:2, col0 : col0 + 128], in_=pview[:, 0:2, :]
                )
                nc.scalar.copy(
                    out=xnT[:, 2:4, col0 : col0 + 128], in_=pview[:, 2:4, :]
                )

            def produce_v(kc, eng=None):
                pv = ps3.tile([128, 3, 512], f32, tag="ps3")
                for cc in range(CCH):
                    nc.tensor.matmul(
                        pv[:, 0, :],
                        xnT[:, cc, kc * 128 : (kc + 1) * 128],
                        w_v[:, cc, :],
                        start=(cc == 0),
                        stop=(cc == CCH - 1),
                    )
                src = pv[:, 0, :].rearrange("p (h d) -> p h d", d=D)
                dst = v_sb[:, kc, :, 0:D]
                if use_b_v:
                    nc.vector.tensor_add(
                        out=dst, in0=src, in1=bv_t.rearrange("p (h d) -> p h d", d=D)
                    )
                elif eng is None:
                    nc.scalar.copy(out=dst, in_=src)
                else:
                    eng.tensor_copy(out=dst, in_=src)

            def produce_kT(pair, rc, on_pav=False, evac=None):
                kT = kTs[pair]
                if on_pav:
                    pk = psum_av.tile([128, 512], f32, tag="pav", name="pkv")
                else:
                    pk3 = ps3.tile([128, 3, 512], f32, tag="ps3", name="pk3")
                    pk = pk3[:, 0, :]
                for cc in range(CCH):
                    nc.tensor.matmul(
                        pk,
                        wk(pair, cc),
                        xnT[:, cc, rc * 512 : (rc + 1) * 512],
                        start=(cc == 0),
                        stop=(cc == CCH - 1),
                    )
                if evac is None:
                    nc.vector.tensor_copy(
                        out=kT[:, rc * 512 : (rc + 1) * 512], in_=pk
                    )
                else:
                    evac.copy(out=kT[:, rc * 512 : (rc + 1) * 512], in_=pk)

            def produce_qT(pair, rc, on_pav=False):
                qT = qTs[pair]
                if on_pav:
                    pq = psum_av.tile([128, 512], f32, tag="pav", name="pqv")
                else:
                    pq3 = ps3.tile([128, 3, 512], f32, tag="ps3", name="pq3")
                    pq = pq3[:, 0, :]
                for cc in range(CCH):
                    nc.tensor.matmul(
                        pq,
                        wq(pair, cc),
                        xnT[:, cc, rc * 512 : (rc + 1) * 512],
                        start=(cc == 0),
                        stop=(cc == CCH - 1),
                    )
                if use_b_q:
                    nc.vector.tensor_scalar_add(
                        out=qT[:, rc * 512 : (rc + 1) * 512],
                        in0=pq,
                        scalar1=bq_t[:, pair, :],
                    )
                else:
                    nc.vector.tensor_copy(
                        out=qT[:, rc * 512 : (rc + 1) * 512], in_=pq
                    )

            # ---- attention stream machinery ----
            class AttState:
                def __init__(self):
                    self.group = None  # (tile, pos, pending)
                    self.full = None
                    self.exp_of = {}
                    self.pavs = None
                    self.av_next = 0
                    self.fl = 0

            def do_flush(st):
                tile_, pos, pending = st.full
                ex = expT_pool.tile([128, 3, 512], bf16, tag="expT")
                nc.scalar.activation(
                    out=ex[:, 0:pos, :],
                    in_=tile_[:, 0:pos, :],
                    func=AF.Exp,
                    scale=SCALE,
                )
                for key, p in pending:
                    st.exp_of[key] = (ex, p)
                st.full = None
                st.fl += 1

            def emit_slice(st, pair, qb, kc, h_idx):
                if st.group is None:
                    if st.full is not None:
                        do_flush(st)
                    st.group = [
                        ps3.tile([128, 3, 512], f32, tag="ps3", name="grp"),
                        0,
                        [],
                    ]
                g = st.group
                base = h_idx * 64
                kT, qT = kTs[pair], qTs[pair]
                nc.tensor.matmul(
                    g[0][:, g[1], :],
                    kT[base : base + 64, kc * 128 : (kc + 1) * 128],
                    qT[base : base + 64, qb * 512 : (qb + 1) * 512],
                )
                g[2].append(((kc, h_idx), g[1]))
                g[1] += 1
                if g[1] == 3:
                    st.full = (g[0], 3, g[2])
                    st.group = None

            def av_drain(st, pair, cap=None, upto=None):
                if st.pavs is None:
                    pav_lo = psum_av.tile([128, 512], f32, tag="pav")
                    pav_hi = psum_av.tile([128, 512], f32, tag="pav")
                    st.pavs = (pav_lo, pav_hi)
                n = 0
                while (
                    st.av_next < KC
                    and (st.av_next, 0) in st.exp_of
                    and (st.av_next, 1) in st.exp_of
                ):
                    if upto is not None and st.av_next >= upto:
                        break
                    if cap is not None and n >= cap:
                        break
                    kc = st.av_next
                    for h_idx in range(2):
                        ex, p = st.exp_of.pop((kc, h_idx))
                        nc.tensor.matmul(
                            st.pavs[h_idx][0 : D + 1, :],
                            v_sb[:, kc, 2 * pair + h_idx, :],
                            ex[:, p, :],
                            start=(kc == 0),
                            stop=(kc == KC - 1),
                        )
                    st.av_next += 1
                    n += 1

            def run_stream(
                st, pair, qb, kcs, extras=None, av_start=0, av_cap=None,
                av_upto=None, flush_end=False,
            ):
                extras = dict(extras or {})
                for kc in kcs:
                    fl0 = st.fl
                    emit_slice(st, pair, qb, kc, 0)
                    emit_slice(st, pair, qb, kc, 1)
                    if st.full is not None:
                        do_flush(st)
                    if st.fl > fl0:
                        if st.fl in extras:
                            extras.pop(st.fl)()
                        if st.fl >= av_start:
                            cap = (
                                av_cap[st.fl % 2]
                                if isinstance(av_cap, tuple)
                                else av_cap
                            )
                            av_drain(st, pair, cap=cap, upto=av_upto)
                for k in sorted(extras):
                    extras.pop(k)()
                if flush_end and st.group is not None:
                    # never leave a partial group holding a ps3 ring slot
                    # across other ps3 users (prefix transposes/kT/v)
                    st.full = (st.group[0], st.group[1], st.group[2])
                    st.group = None
                    do_flush(st)
                    if st.fl >= av_start:
                        cap = (
                            av_cap[st.fl % 2] if isinstance(av_cap, tuple) else av_cap
                        )
                        av_drain(st, pair, cap=cap, upto=av_upto)

            def close_stream(st, pair):
                if st.group is not None:
                    st.full = (st.group[0], st.group[1], st.group[2])
                    st.group = None
                if st.full is not None:
                    do_flush(st)
                av_drain(st, pair)
                assert st.av_next == KC, st.av_next
                vts = []
                for h_idx in range(2):
                    vt = valT_pool.tile([D + 1, 512], f32, tag="valT", bufs=4)
                    nc.vector.tensor_copy(out=vt, in_=st.pavs[h_idx][0 : D + 1, :])
                    vts.append(vt)
                st.pavs = None
                return vts

            def transpose_half(vts, qb, pair, half, on_pav=False):
                h = 2 * pair + half
                if on_pav:
                    bank = psum_av.tile([128, 512], f32, tag="pav", name="tsc")
                else:
                    pt = ps3.tile([128, 3, 512], f32, tag="ps3")
                    bank = pt[:, 0, :]
                pv = bank.rearrange("p (j d) -> p j d", d=128)
                for j in range(4):
                    nc.tensor.transpose(
                        pv[:, j, 0 : D + 1],
                        vts[half][:, j * 128 : (j + 1) * 128],
                        ident[0 : D + 1, 0 : D + 1],
                    )
                nc.vector.tensor_copy(
                    out=val_asm[:, qb * 4 : qb * 4 + 4, h, :],
                    in_=pv[:, :, 0 : D + 1],
                )

            # ---- phase 5 per qb half ----
            class Ph5:
                def __init__(self, qtiles):
                    self.qtiles = qtiles
                    self.ots = []
                    self.m4 = stats.tile([128, 4], f32, tag="m4b", name="m4")
                    self.a4b = stats.tile([128, 4], f32, tag="a4b", name="a4b")
                    self.r4 = stats.tile([128, 4], f32, tag="r4b", name="r4")

                def stats_one(self, jj, eng=None, mul_act=False):
                    eng = eng or nc.vector
                    qtile = self.qtiles[jj]
                    va = val_asm[:, qtile]
                    ot = out_pool.tile([128, C], f32, tag="ot", name="ot")
                    rs8 = stats.tile([128, 8], f32, tag="rs8", name="rs8")
                    nc.vector.reciprocal(
                        out=rs8,
                        in_=va[:, :, D : D + 1].rearrange("p h one -> p (h one)"),
                    )
                    for h in range(H):
                        if mul_act:
                            nc.scalar.activation(
                                out=ot[:, h * D : (h + 1) * D],
                                in_=va[:, h, 0:D],
                                func=AF.Identity,
                                scale=rs8[:, h : h + 1],
                            )
                        else:
                            eng.tensor_scalar_mul(
                                out=ot[:, h * D : (h + 1) * D],
                                in0=va[:, h, 0:D],
                                scalar1=rs8[:, h : h + 1],
                            )
                    if use_b_v:
                        eng.tensor_add(out=ot, in0=ot, in1=bv_t)
                    st6 = stats.tile([128, 6], f32, tag="bn6", name="st6")
                    nc.vector.bn_stats(out=st6, in_=ot)
                    mv = stats.tile([128, 2], f32, tag="mv", name="mv")
                    nc.vector.bn_aggr(out=mv, in_=st6)
                    nc.vector.tensor_copy(out=self.m4[:, jj : jj + 1], in_=mv[:, 0:1])
                    nc.vector.tensor_copy(out=self.a4b[:, jj : jj + 1], in_=mv[:, 1:2])
                    self.ots.append(ot)

                def rsqrt(self):
                    nc.vector.tensor_scalar_add(out=self.a4b, in0=self.a4b, scalar1=EPS)
                    rsqrt_into(self.r4, self.a4b, 4, "p5")
                    self.nmr = stats.tile([128, 4], f32, tag="nmr", name="nmr")
                    nc.vector.tensor_mul(out=self.nmr, in0=self.m4, in1=self.r4)
                    nc.vector.tensor_scalar_mul(out=self.nmr, in0=self.nmr, scalar1=-1.0)

                def fin_one(self, jj, eng=None, use_act=False):
                    eng = eng or nc.vector
                    qtile = self.qtiles[jj]
                    ot = self.ots[jj]
                    if use_act:
                        # (x - m)*r on ScalarE (idle at the tail): x*r - m*r
                        nc.scalar.activation(
                            out=ot,
                            in_=ot,
                            func=AF.Identity,
                            scale=self.r4[:, jj : jj + 1],
                            bias=self.nmr[:, jj : jj + 1],
                        )
                    else:
                        eng.tensor_scalar(
                            out=ot,
                            in0=ot,
                            scalar1=self.m4[:, jj : jj + 1],
                            scalar2=self.r4[:, jj : jj + 1],
                            op0=ALU.subtract,
                            op1=ALU.mult,
                        )
                    if use_g_post:
                        eng.tensor_mul(out=ot, in0=ot, in1=g_post_t)
                    if use_beta_post:
                        eng.tensor_add(out=ot, in0=ot, in1=beta_post_t)
                    eng.tensor_add(out=ot, in0=ot, in1=xn_rows[:, qtile, :])
                    nc.sync.dma_start(
                        out=out[qtile * 128 : (qtile + 1) * 128, :], in_=ot
                    )

            # ================= emission =================
            states = [[AttState() for _ in range(NQB)] for _ in range(NPAIR)]
            st00 = states[0][0]

            # phase 1+2: LN -> xnT transposes -> kT0/qT0 -> v (lagged),
            # with the pair0-qb0 score/exp/AV stream trickling one rc behind
            for rc in range(NT // 4):
                if rc >= 1:
                    run_stream(
                        st00, 0, 0, range(4 * (rc - 1), 4 * rc),
                        av_upto=4 * (rc - 1), flush_end=True,
                    )
                    # v for kc [4(rc-1), 4rc): xnT inputs are from iteration
                    # rc-1, so these MMs fill the PE idle window while this
                    # iteration's LN chain runs on DVE (keeps HAM warm)
                    for kc in range(4 * (rc - 1), 4 * rc):
                        produce_v(kc)
                xts, mvs = [], []
                for j in range(4):
                    i = 4 * rc + j
                    xt = ln_in.tile([128, C], f32, tag="xt")
                    nc.sync.dma_start(out=xt, in_=xb[i * 128 : (i + 1) * 128, :])
                    st6 = stats.tile([128, 6], f32, tag="bn6")
                    nc.vector.bn_stats(out=st6, in_=xt)
                    mv = stats.tile([128, 2], f32, tag="mv")
                    nc.vector.bn_aggr(out=mv, in_=st6)
                    xts.append(xt)
                    mvs.append(mv)
                a4 = stats.tile([128, 4], f32, tag="a4")
                for j in range(4):
                    nc.vector.tensor_copy(out=a4[:, j : j + 1], in_=mvs[j][:, 1:2])
                nc.vector.tensor_scalar_add(out=a4, in0=a4, scalar1=EPS)
                r4 = stats.tile([128, 4], f32, tag="r4")
                rsqrt_into(r4, a4, 4, "p1")
                for j in range(4):
                    i = 4 * rc + j
                    eng = nc.vector
                    xbf = ln_in.tile([128, C], bf16, tag="xbf", bufs=4)
                    if i < QT or use_g_pre or use_beta_pre:
                        dst = xn_rows[:, i, :] if i < QT else xts[j]
                        eng.tensor_scalar(
                            out=dst,
                            in0=xts[j],
                            scalar1=mvs[j][:, 0:1],
                            scalar2=r4[:, j : j + 1],
                            op0=ALU.subtract,
                            op1=ALU.mult,
                        )
                        if use_g_pre:
                            eng.tensor_mul(out=dst, in0=dst, in1=g_pre_t)
                        if use_beta_pre:
                            eng.tensor_add(out=dst, in0=dst, in1=beta_pre_t)
                        eng.tensor_copy(out=xbf, in_=dst)
                    else:
                        eng.tensor_scalar(
                            out=xbf,
                            in0=xts[j],
                            scalar1=mvs[j][:, 0:1],
                            scalar2=r4[:, j : j + 1],
                            op0=ALU.subtract,
                            op1=ALU.mult,
                        )
                    transpose_into(xbf, i * 128)
                produce_kT(0, rc)
                if rc < NQB:
                    produce_qT(0, rc)
                # late prefix: next pair's kT ch0-3 on the ring
                if rc >= 6:
                    for r2 in range(2 * (rc - 6), 2 * (rc - 6) + 2):
                        produce_kT(1, r2)
                if rc == 0:
                    # bulk weight DMAs, after the first x tiles are queued
                    nc.sync.dma_start(out=w_v, in_=wv_d)
                    nc.sync.dma_start(out=w_k1, in_=wk1_d)
                    nc.sync.dma_start(out=w_q1, in_=wq1_d)
                    if use_b_v:
                        bv_t = consts.tile([128, C], f32)
                        nc.sync.dma_start(
                            out=bv_t, in_=bcast(bqkv[2 * C : 3 * C], 128)
                        )
                    if use_g_post:
                        g_post_t = consts.tile([128, C], f32)
                        nc.sync.dma_start(out=g_post_t, in_=bcast(g_post, 128))
                    if use_beta_post:
                        beta_post_t = consts.tile([128, C], f32)
                        nc.sync.dma_start(
                            out=beta_post_t, in_=bcast(beta_post, 128)
                        )
                if 2 <= rc <= 4:
                    # qT pair rc-1, both chunks in ONE ring slot
                    pq2 = ps3.tile([128, 3, 512], f32, tag="ps3", name="pq2")
                    for i in range(2):
                        for cc in range(CCH):
                            nc.tensor.matmul(
                                pq2[:, i, :],
                                wq(rc - 1, cc),
                                xnT[:, cc, i * 512 : (i + 1) * 512],
                                start=(cc == 0),
                                stop=(cc == CCH - 1),
                            )
                    for i in range(2):
                        if use_b_q:
                            nc.vector.tensor_scalar_add(
                                out=qTs[rc - 1][:, i * 512 : (i + 1) * 512],
                                in0=pq2[:, i, :],
                                scalar1=bq_t[:, rc - 1, :],
                            )
                        else:
                            nc.vector.tensor_copy(
                                out=qTs[rc - 1][:, i * 512 : (i + 1) * 512],
                                in_=pq2[:, i, :],
                            )

            # stream0 tail: last v chunks, remaining scores, full AV drain
            for kc in range(28, 32):
                produce_v(kc, eng=nc.vector)
            run_stream(st00, 0, 0, range(28, 32))
            vts_prev = close_stream(st00, 0)
            tp_prev = (0, 0)

            # streams 1..7, qb-interleaved: (p0,q1),(p1,q0),(p1,q1),...
            order = [(0, 1), (1, 0), (1, 1), (2, 0), (2, 1), (3, 0), (3, 1)]
            ph5a = Ph5([0, 1, 2, 3])
            for pair, qb in order:
                st = states[pair][qb]
                pp, pqb = tp_prev
                vp = vts_prev
                ex = {
                    1: (lambda vp=vp, pqb=pqb, pp=pp: transpose_half(vp, pqb, pp, 0, on_pav=True)),
                    2: (lambda vp=vp, pqb=pqb, pp=pp: transpose_half(vp, pqb, pp, 1, on_pav=True)),
                }
                av_start = 6
                if qb == 0 and pair + 1 < NPAIR:
                    for t in range(4):
                        ex[3 + t] = (
                            lambda pair=pair, t=t: produce_kT(pair + 1, t, on_pav=True)
                        )
                elif qb == 1 and pair + 1 < NPAIR:
                    for t in range(4):
                        ex[3 + t] = (
                            lambda pair=pair, t=t: produce_kT(
                                pair + 1, 4 + t, on_pav=True
                            )
                        )
                if (pair, qb) == (3, 1):
                    # phase5 for the qb0 rows rides the last stream
                    ex.update(
                        {
                            4: lambda: ph5a.stats_one(0),
                            6: lambda: ph5a.stats_one(1),
                            8: lambda: ph5a.stats_one(2),
                            10: lambda: ph5a.stats_one(3),
                            12: lambda: ph5a.rsqrt(),
                            14: lambda: ph5a.fin_one(0),
                            16: lambda: ph5a.fin_one(1),
                            18: lambda: ph5a.fin_one(2),
                            20: lambda: ph5a.fin_one(3),
                        }
                    )
                run_stream(
                    st, pair, qb, range(KC),
                    extras=ex, av_start=av_start, av_cap=(5, 4),
                )
                vts_prev = close_stream(st, pair)
                tp_prev = (pair, qb)

            # tail: last vt transposes + phase5 for qb1, DVE/Pool pipelined
            transpose_half(vts_prev, tp_prev[1], tp_prev[0], 0, on_pav=True)
            transpose_half(vts_prev, tp_prev[1], tp_prev[0], 1, on_pav=True)
            ph5b = Ph5([4, 5, 6, 7])
            ph5b.stats_one(0, mul_act=True)
            ph5b.stats_one(1)
            ph5b.stats_one(2, mul_act=True)
            ph5b.stats_one(3)
            ph5b.rsqrt()
            ph5b.fin_one(0, use_act=True)
            ph5b.fin_one(1)
            ph5b.fin_one(2, use_act=True)
            ph5b.fin_one(3)

    nc.compile()
    return nc


def kernel(x, w_qkv, b_qkv, g_pre, beta_pre, g_post, beta_post):
    import ml_dtypes
    from concourse.bass_utils import run_bass_kernel_spmd

    x = np.asarray(x, dtype=np.float32)
    w_qkv = np.asarray(w_qkv, dtype=np.float32)
    b_qkv = np.asarray(b_qkv, dtype=np.float32)
    g_pre = np.asarray(g_pre, dtype=np.float32)
    beta_pre = np.asarray(beta_pre, dtype=np.float32)
    g_post = np.asarray(g_post, dtype=np.float32)
    beta_post = np.asarray(beta_post, dtype=np.float32)

    flags = (
        not np.all(g_pre == 1.0),
        not np.all(beta_pre == 0.0),
        not np.all(g_post == 1.0),
        not np.all(beta_post == 0.0),
        not np.all(b_qkv[0:C] == 0.0),
        not np.all(b_qkv[2 * C : 3 * C] == 0.0),
    )
    # NOTE: b_qkv[C:2C] (the K bias) provably cancels in softmax and is
    # intentionally never applied.
    if flags not in _CACHE:
        _CACHE[flags] = _build(flags)
    nc = _CACHE[flags]

    w_bf = w_qkv.astype(ml_dtypes.bfloat16)
    wt = w_bf.reshape(4, 128, 3 * C)  # (cc, p, m)

    def blk(lo, hi):
        return np.ascontiguousarray(wt[:, :, lo:hi].transpose(1, 0, 2))

    wk0 = blk(C, C + 128)
    wq0 = blk(0, 128)
    wv = blk(2 * C, 3 * C)
    wk1 = blk(C + 128, 2 * C)
    wq1 = blk(128, C)
    in_maps = []
    for c in range(8):
        b = c // 4
        r = c % 4
        xrot = np.ascontiguousarray(
            np.concatenate([x[b, r * QR :], x[b, : r * QR]], axis=0)
        )
        in_maps.append(
            {
                "xb": xrot,
                "wk0": wk0,
                "wq0": wq0,
                "wv": wv,
                "wk1": wk1,
                "wq1": wq1,
                "b_qkv": b_qkv,
                "g_pre": g_pre,
                "beta_pre": beta_pre,
                "g_post": g_post,
                "beta_post": beta_post,
            }
        )

    global _last_in_maps
    _last_in_maps = in_maps
    res = run_bass_kernel_spmd(nc, in_maps, core_ids=list(range(8)))
    out = np.empty((B, N, C), dtype=np.float32)
    for c in range(8):
        b = c // 4
        r = c % 4
        out[b, r * QR : (r + 1) * QR] = res.results[c]["out"]
    return out
